# revision 1
# baseline (speedup 1.0000x reference)
"""GroupSparseAE (FISTA group-lasso encoder + linear decoder) on 8 trn2 cores.

Data-parallel over batch: each core gets B/8 = 64 rows, W replicated.
Per channel c (3 total, processed sequentially so W[c]/W[c]^T fit in SBUF):
  y2   = TAU * (W @ x^T)                   [D, b] transposed layout
  FISTA iterate k = 1..30 with x in transposed [D, b] layout:
    u^T    = W^T-contract:  uT[n,b]   = sum_d W[d,n] xT[d,b]
    grad^T = gT[e,b]        = sum_n WT[n,e] uT[n,b]
    v      = xT_tmp + y2 - TAU*gT
    group soft-threshold (groups of 8 along d = partition dim):
       gs = Bmat^T @ v^2  (Bmat block-diag ones -> broadcast group sumsq)
       xnew = relu(v) * relu(1 - c/sqrt(gs))
    momentum: xtmp = xnew + m_k (xnew - xold)
  decode: out^T[n,b] = sum_d W[d,n] z[d,b]
All matmuls: stationary [128,128] weight tile, moving [128,64] activation
slice, fp32 accumulate in PSUM.
"""

import sys

sys.path.insert(0, "/opt/trn_rl_repo")

import numpy as np

B, C, N = 512, 3, 1024
G, S = 256, 8
D = G * S  # 2048
NUM_LAYERS = 30
TAU, LAM = 0.1, 0.1
CTH = LAM * TAU  # group threshold constant

N_CORES = 8
BL = B // N_CORES  # 64 rows per core
NT = D // 128  # 16 d-tiles
NS = N // 128  # 8 n-tiles
FD = NT * BL  # 1024 flat free dim of [D, b] state
CHUNK = 256  # elementwise chunk (4 d-tiles)
NCH = FD // CHUNK


def _mom_coeffs(num_layers):
    # fp32 t-sequence to match the reference's on-device arithmetic
    one, four, two = np.float32(1.0), np.float32(4.0), np.float32(2.0)
    t = np.float32(1.0)
    ms = []
    for _ in range(num_layers):
        t_new = (one + np.sqrt(one + four * t * t)) / two
        ms.append(float((t - one) / t_new))
        t = t_new
    return ms


def _bmat_np():
    p = np.arange(128)
    return (p[:, None] // S == p[None, :] // S).astype(np.float32)


def build(num_layers=NUM_LAYERS):
    import concourse.bacc as bacc
    from concourse import mybir
    from concourse.tile import TileContext

    fp32 = mybir.dt.float32
    AF = mybir.ActivationFunctionType
    OP = mybir.AluOpType

    nc = bacc.Bacc("TRN2", target_bir_lowering=False, debug=False,
                   num_devices=N_CORES)
    xt = nc.dram_tensor("xt", [C, N, BL], fp32, kind="ExternalInput")
    w = nc.dram_tensor("w", [C, D, N], fp32, kind="ExternalInput")
    wt = nc.dram_tensor("wt", [C, N, D], fp32, kind="ExternalInput")
    bm = nc.dram_tensor("bm", [128, 128], fp32, kind="ExternalInput")
    ot = nc.dram_tensor("ot", [C, N, BL], fp32, kind="ExternalOutput")

    ms = _mom_coeffs(num_layers)

    with TileContext(nc) as tc:
        with (
            tc.tile_pool(name="wp", bufs=1) as wp,
            tc.tile_pool(name="st", bufs=1) as st,
            tc.tile_pool(name="scr", bufs=4) as scr,
            tc.tile_pool(name="ps_u", bufs=2, space="PSUM") as ps_u,
            tc.tile_pool(name="ps_g", bufs=3, space="PSUM") as ps_g,
            tc.tile_pool(name="ps_s", bufs=2, space="PSUM") as ps_s,
        ):
            bmat = wp.tile([128, 128], fp32, tag="bmat")
            nc.sync.dma_start(out=bmat, in_=bm[:, :])

            for c in range(C):
                wsb = wp.tile([128, NT, N], fp32, tag="wsb")
                nc.sync.dma_start(
                    out=wsb, in_=w[c].rearrange("(t p) n -> p t n", p=128))
                wtsb = wp.tile([128, NS, D], fp32, tag="wtsb")
                nc.sync.dma_start(
                    out=wtsb, in_=wt[c].rearrange("(s p) e -> p s e", p=128))
                xts = wp.tile([128, NS, BL], fp32, tag="xts")
                nc.sync.dma_start(
                    out=xts, in_=xt[c].rearrange("(s p) b -> p s b", p=128))

                # persistent per-channel state
                y2 = st.tile([128, FD], fp32, tag="y2")
                xb0 = st.tile([128, FD], fp32, tag="xb0")
                xb1 = st.tile([128, FD], fp32, tag="xb1")
                xbuf = [xb0, xb1]
                uT = st.tile([128, NS * BL], fp32, tag="uT")
                # chunked tiles for cross-iteration pipelining
                xtmp = [st.tile([128, CHUNK], fp32, tag=f"xtmp{j}",
                                name=f"xtmp{j}") for j in range(NCH)]
                pre = [st.tile([128, CHUNK], fp32, tag=f"pre{j}",
                               name=f"pre{j}") for j in range(NCH)]

                nc.vector.memset(xb0, 0.0)

                # ---- precomp: y2 = TAU * W @ x^T  in [D, b] layout ----
                for t in range(NT):
                    py = ps_g.tile([128, BL], fp32, tag="pg")
                    for s in range(NS):
                        nc.tensor.matmul(
                            py, wtsb[:, s, t * 128:(t + 1) * 128],
                            xts[:, s, :], start=(s == 0), stop=(s == NS - 1))
                    nc.scalar.mul(y2[:, t * BL:(t + 1) * BL], py, TAU)

                def act_block(vch, k):
                    """vch(j) -> [128, CHUNK] AP of the pre-activation v.
                    Writes xnew (xbuf[k % 2]); unless last iter, also xtmp/pre.
                    """
                    xnew, xold = xbuf[k % 2], xbuf[(k - 1) % 2]
                    m = ms[k - 1]
                    last = k == num_layers
                    for j in range(NCH):
                        sl = slice(j * CHUNK, (j + 1) * CHUNK)
                        vj = vch(j)
                        v2 = scr.tile([128, CHUNK], fp32, tag="v2")
                        nc.scalar.square(v2, vj)
                        gs = ps_s.tile([128, CHUNK], fp32, tag="gs")
                        nc.tensor.matmul(gs, bmat, v2, start=True, stop=True)
                        nrm = scr.tile([128, CHUNK], fp32, tag="nrm")
                        nc.scalar.sqrt(nrm, gs)
                        invn = scr.tile([128, CHUNK], fp32, tag="invn")
                        nc.vector.reciprocal(invn, nrm)
                        scl = scr.tile([128, CHUNK], fp32, tag="scl")
                        # relu(1 - CTH / nrm)
                        nc.scalar.activation(scl, invn, AF.Relu,
                                             bias=1.0, scale=-CTH)
                        # xnew = max(v, 0) * scl
                        nc.vector.scalar_tensor_tensor(
                            xnew[:, sl], vj, 0.0, scl,
                            op0=OP.max, op1=OP.mult)
                        if not last:
                            dd = scr.tile([128, CHUNK], fp32, tag="dd")
                            nc.vector.tensor_sub(dd, xnew[:, sl], xold[:, sl])
                            nc.vector.scalar_tensor_tensor(
                                xtmp[j], dd, m, xnew[:, sl],
                                op0=OP.mult, op1=OP.add)
                            nc.vector.tensor_add(pre[j], xtmp[j], y2[:, sl])

                # ---- iteration 1: x_tmp = 0 -> v = y2 ----
                act_block(lambda j: y2[:, j * CHUNK:(j + 1) * CHUNK], 1)

                # ---- iterations 2..num_layers ----
                for k in range(2, num_layers + 1):
                    # u-phase: uT[n,b] = sum_d W[d,n] xtmp[d,b]
                    for s in range(NS):
                        pu = ps_u.tile([128, BL], fp32, tag="pu")
                        for t in range(NT):
                            nc.tensor.matmul(
                                pu, wsb[:, t, s * 128:(s + 1) * 128],
                                xtmp[t // 4][:, (t % 4) * BL:(t % 4 + 1) * BL],
                                start=(t == 0), stop=(t == NT - 1))
                        nc.scalar.copy(uT[:, s * BL:(s + 1) * BL], pu)
                    # grad-phase + v-combine
                    vt = [scr.tile([128, CHUNK], fp32, tag=f"v{j}", name=f"v{j}")
                          for j in range(NCH)]
                    for t in range(NT):
                        pg = ps_g.tile([128, BL], fp32, tag="pg")
                        for s in range(NS):
                            nc.tensor.matmul(
                                pg, wtsb[:, s, t * 128:(t + 1) * 128],
                                uT[:, s * BL:(s + 1) * BL],
                                start=(s == 0), stop=(s == NS - 1))
                        # v = pre - TAU * grad
                        nc.vector.scalar_tensor_tensor(
                            vt[t // 4][:, (t % 4) * BL:(t % 4 + 1) * BL],
                            pg, -TAU, pre[t // 4][:, (t % 4) * BL:(t % 4 + 1) * BL],
                            op0=OP.mult, op1=OP.add)
                    act_block(lambda j: vt[j][:, :], k)

                # ---- decode: out^T[n,b] = sum_d W[d,n] z[d,b] ----
                z = xbuf[num_layers % 2]
                otsb = st.tile([128, NS, BL], fp32, tag="otsb")
                for s in range(NS):
                    pd = ps_u.tile([128, BL], fp32, tag="pu")
                    for t in range(NT):
                        nc.tensor.matmul(
                            pd, wsb[:, t, s * 128:(s + 1) * 128],
                            z[:, t * BL:(t + 1) * BL],
                            start=(t == 0), stop=(t == NT - 1))
                    nc.scalar.copy(otsb[:, s, :], pd)
                nc.sync.dma_start(
                    out=ot[c].rearrange("(s p) b -> p s b", p=128), in_=otsb)

    nc.compile()
    return nc


_CACHED = {}


def _get_nc(num_layers=NUM_LAYERS):
    if num_layers not in _CACHED:
        _CACHED[num_layers] = build(num_layers)
    return _CACHED[num_layers]


def make_in_maps(x, w):
    """x [B,C,N] fp32, w [C,D,N] fp32 -> list of 8 per-core input dicts."""
    x = np.asarray(x, dtype=np.float32)
    w = np.ascontiguousarray(np.asarray(w, dtype=np.float32))
    wt = np.ascontiguousarray(w.transpose(0, 2, 1))
    bm = _bmat_np()
    maps = []
    for i in range(N_CORES):
        xs = x[i * BL:(i + 1) * BL]  # [BL, C, N]
        xts = np.ascontiguousarray(xs.transpose(1, 2, 0))  # [C, N, BL]
        maps.append({"xt": xts, "w": w, "wt": wt, "bm": bm})
    return maps


def assemble_out(results):
    outs = []
    for i in range(N_CORES):
        o = results[i]["ot"]  # [C, N, BL]
        outs.append(np.ascontiguousarray(o.transpose(2, 0, 1)))  # [BL, C, N]
    return np.concatenate(outs, axis=0).astype(np.float32)


def kernel(x, W):
    from concourse.bass_utils import run_bass_kernel_spmd

    nc = _get_nc()
    res = run_bass_kernel_spmd(nc, make_in_maps(x, W), list(range(N_CORES)))
    return assemble_out(res.results)


if __name__ == "__main__":
    xs = np.random.randn(B, C, N).astype(np.float32)
    ws = np.random.randn(C, D, N).astype(np.float32)
    ws /= np.linalg.norm(ws, axis=-1, keepdims=True)
    out = kernel(xs, ws)
    print("out", out.shape, out.dtype, float(np.abs(out).mean()))



# revision 3
# speedup vs baseline: 4.6499x; 4.6499x over previous
"""GroupSparseAE (FISTA group-lasso encoder + linear decoder) on 8 trn2 cores.

Data-parallel over batch: each core gets B/8 = 64 rows, W replicated.
Per channel c (3 total, processed sequentially so W[c]/W[c]^T fit in SBUF):
  y2   = TAU * (W @ x^T)                   [D, b] transposed layout
  FISTA iterate k = 1..30 with x in transposed [D, b] layout:
    u^T    = W^T-contract:  uT[n,b]   = sum_d W[d,n] xT[d,b]
    grad^T = gT[e,b]        = sum_n WT[n,e] uT[n,b]
    v      = xT_tmp + y2 - TAU*gT
    group soft-threshold (groups of 8 along d = partition dim):
       gs = Bmat^T @ v^2  (Bmat block-diag ones -> broadcast group sumsq)
       xnew = relu(v) * relu(1 - c*rsqrt(gs))
    momentum: xtmp = xnew + m_k (xnew - xold)
  decode: out^T[n,b] = sum_d W[d,n] z[d,b]

Precision: all matmul operands are bf16 (4x faster PE than fp32 which runs
as 2 half-speed LOW/HIGH passes); the FISTA state (xnew/xold/pre/y2) stays
fp32 so quantization error does not accumulate across the 30 iterations
(numpy-simulated rel err 3.7e-3 vs 1.7e-2 with bf16 state).
All matmuls: stationary [128,128] bf16 weight tile, moving [128,64] bf16
activation slice, fp32 accumulate in PSUM.
"""

import sys

sys.path.insert(0, "/opt/trn_rl_repo")

import numpy as np

B, C, N = 512, 3, 1024
G, S = 256, 8
D = G * S  # 2048
NUM_LAYERS = 30
TAU, LAM = 0.1, 0.1
CTH = LAM * TAU  # group threshold constant

N_CORES = 8
BL = B // N_CORES  # 64 rows per core
NT = D // 128  # 16 d-tiles
NS = N // 128  # 8 n-tiles
FD = NT * BL  # 1024 flat free dim of [D, b] state
CHUNK = 256  # elementwise chunk (4 d-tiles)
NCH = FD // CHUNK

USE_GPSIMD = True  # offload dd/tmp elementwise ops to the idle gpsimd engine


def _mom_coeffs(num_layers):
    # fp32 t-sequence to match the reference's on-device arithmetic
    one, four, two = np.float32(1.0), np.float32(4.0), np.float32(2.0)
    t = np.float32(1.0)
    ms = []
    for _ in range(num_layers):
        t_new = (one + np.sqrt(one + four * t * t)) / two
        ms.append(float((t - one) / t_new))
        t = t_new
    return ms


def _bmat_np():
    p = np.arange(128)
    return (p[:, None] // S == p[None, :] // S).astype(np.float32)


def build(num_layers=NUM_LAYERS):
    import concourse.bacc as bacc
    from concourse import mybir
    from concourse.tile import TileContext

    fp32 = mybir.dt.float32
    bf16 = mybir.dt.bfloat16
    AF = mybir.ActivationFunctionType
    OP = mybir.AluOpType

    nc = bacc.Bacc("TRN2", target_bir_lowering=False, debug=False,
                   num_devices=N_CORES)
    xt = nc.dram_tensor("xt", [C, N, BL], bf16, kind="ExternalInput")
    w = nc.dram_tensor("w", [C, D, N], bf16, kind="ExternalInput")
    wt = nc.dram_tensor("wt", [C, N, D], bf16, kind="ExternalInput")
    bm = nc.dram_tensor("bm", [128, 128], bf16, kind="ExternalInput")
    ot = nc.dram_tensor("ot", [C, N, BL], fp32, kind="ExternalOutput")

    ms = _mom_coeffs(num_layers)

    with TileContext(nc) as tc:
        with (
            tc.tile_pool(name="wp", bufs=2) as wp,
            tc.tile_pool(name="st", bufs=1) as st,
            tc.tile_pool(name="scr", bufs=4) as scr,
            tc.tile_pool(name="ps_u", bufs=2, space="PSUM") as ps_u,
            tc.tile_pool(name="ps_g", bufs=3, space="PSUM") as ps_g,
            tc.tile_pool(name="ps_s", bufs=2, space="PSUM") as ps_s,
        ):
            bmat = wp.tile([128, 128], bf16, tag="bmat")
            nc.sync.dma_start(out=bmat, in_=bm[:, :])

            for c in range(C):
                wsb = wp.tile([128, NT, N], bf16, tag="wsb")
                nc.sync.dma_start(
                    out=wsb, in_=w[c].rearrange("(t p) n -> p t n", p=128))
                wtsb = wp.tile([128, NS, D], bf16, tag="wtsb")
                nc.sync.dma_start(
                    out=wtsb, in_=wt[c].rearrange("(s p) e -> p s e", p=128))
                xts = wp.tile([128, NS, BL], bf16, tag="xts")
                nc.sync.dma_start(
                    out=xts, in_=xt[c].rearrange("(s p) b -> p s b", p=128))

                # persistent per-channel state (fp32 except matmul inputs)
                y2 = st.tile([128, FD], fp32, tag="y2")
                xb0 = st.tile([128, FD], fp32, tag="xb0")
                xb1 = st.tile([128, FD], fp32, tag="xb1")
                xbuf = [xb0, xb1]
                uT = st.tile([128, NS * BL], bf16, tag="uT")
                # chunked tiles for cross-iteration pipelining
                xtmp = [st.tile([128, CHUNK], bf16, tag=f"xtmp{j}",
                                name=f"xtmp{j}") for j in range(NCH)]
                pre = [st.tile([128, CHUNK], fp32, tag=f"pre{j}",
                               name=f"pre{j}") for j in range(NCH)]

                nc.vector.memset(xb0, 0.0)

                # ---- precomp: y2 = TAU * W @ x^T  in [D, b] layout ----
                for t in range(NT):
                    py = ps_g.tile([128, BL], fp32, tag="pg")
                    for s in range(NS):
                        nc.tensor.matmul(
                            py, wtsb[:, s, t * 128:(t + 1) * 128],
                            xts[:, s, :], start=(s == 0), stop=(s == NS - 1))
                    nc.scalar.mul(y2[:, t * BL:(t + 1) * BL], py, TAU)

                def act_block(vch, k):
                    """vch(j) -> [128, CHUNK] AP of the pre-activation v.
                    Writes xnew (xbuf[k % 2]); unless last iter, also
                    xtmp (bf16 matmul input) and pre (fp32 = xtmp + y2).
                    """
                    xnew, xold = xbuf[k % 2], xbuf[(k - 1) % 2]
                    m = ms[k - 1]
                    last = k == num_layers
                    eng2 = nc.gpsimd if USE_GPSIMD else nc.vector
                    for j in range(NCH):
                        sl = slice(j * CHUNK, (j + 1) * CHUNK)
                        vj = vch(j)
                        v2 = scr.tile([128, CHUNK], bf16, tag="v2")
                        nc.scalar.square(v2, vj)
                        gs = ps_s.tile([128, CHUNK], fp32, tag="gs")
                        nc.tensor.matmul(gs, bmat, v2, start=True, stop=True)
                        nrm = scr.tile([128, CHUNK], fp32, tag="nrm")
                        nc.scalar.sqrt(nrm, gs)
                        invn = scr.tile([128, CHUNK], fp32, tag="invn")
                        nc.vector.reciprocal(invn, nrm)
                        scl = scr.tile([128, CHUNK], fp32, tag="scl")
                        # relu(1 - CTH / nrm)
                        nc.scalar.activation(scl, invn, AF.Relu,
                                             bias=1.0, scale=-CTH)
                        # xnew = max(v, 0) * scl
                        nc.vector.scalar_tensor_tensor(
                            xnew[:, sl], vj, 0.0, scl,
                            op0=OP.max, op1=OP.mult)
                        if not last:
                            dd = scr.tile([128, CHUNK], fp32, tag="dd")
                            eng2.tensor_sub(dd, xnew[:, sl], xold[:, sl])
                            # bf16 matmul input: xtmp = xnew + m*dd
                            nc.vector.scalar_tensor_tensor(
                                xtmp[j], dd, m, xnew[:, sl],
                                op0=OP.mult, op1=OP.add)
                            # fp32 identity path: pre = m*dd + (xnew + y2)
                            tmp = scr.tile([128, CHUNK], fp32, tag="tmp")
                            eng2.tensor_add(tmp, xnew[:, sl], y2[:, sl])
                            nc.vector.scalar_tensor_tensor(
                                pre[j], dd, m, tmp,
                                op0=OP.mult, op1=OP.add)

                # ---- iteration 1: x_tmp = 0 -> v = y2 ----
                act_block(lambda j: y2[:, j * CHUNK:(j + 1) * CHUNK], 1)

                # ---- iterations 2..num_layers ----
                for k in range(2, num_layers + 1):
                    # u-phase: uT[n,b] = sum_d W[d,n] xtmp[d,b]
                    for s in range(NS):
                        pu = ps_u.tile([128, BL], fp32, tag="pu")
                        for t in range(NT):
                            nc.tensor.matmul(
                                pu, wsb[:, t, s * 128:(s + 1) * 128],
                                xtmp[t // 4][:, (t % 4) * BL:(t % 4 + 1) * BL],
                                start=(t == 0), stop=(t == NT - 1))
                        nc.scalar.copy(uT[:, s * BL:(s + 1) * BL], pu)
                    # grad-phase + v-combine
                    vt = [scr.tile([128, CHUNK], fp32, tag=f"v{j}", name=f"v{j}")
                          for j in range(NCH)]
                    for t in range(NT):
                        pg = ps_g.tile([128, BL], fp32, tag="pg")
                        for s in range(NS):
                            nc.tensor.matmul(
                                pg, wtsb[:, s, t * 128:(t + 1) * 128],
                                uT[:, s * BL:(s + 1) * BL],
                                start=(s == 0), stop=(s == NS - 1))
                        # v = pre - TAU * grad
                        nc.vector.scalar_tensor_tensor(
                            vt[t // 4][:, (t % 4) * BL:(t % 4 + 1) * BL],
                            pg, -TAU, pre[t // 4][:, (t % 4) * BL:(t % 4 + 1) * BL],
                            op0=OP.mult, op1=OP.add)
                    act_block(lambda j: vt[j][:, :], k)

                # ---- decode: out^T[n,b] = sum_d W[d,n] z[d,b] ----
                z = xbuf[num_layers % 2]
                zbf = st.tile([128, FD], bf16, tag="zbf")
                nc.scalar.copy(zbf, z)
                otsb = st.tile([128, NS, BL], fp32, tag="otsb")
                for s in range(NS):
                    pd = ps_u.tile([128, BL], fp32, tag="pu")
                    for t in range(NT):
                        nc.tensor.matmul(
                            pd, wsb[:, t, s * 128:(s + 1) * 128],
                            zbf[:, t * BL:(t + 1) * BL],
                            start=(t == 0), stop=(t == NT - 1))
                    nc.scalar.copy(otsb[:, s, :], pd)
                nc.sync.dma_start(
                    out=ot[c].rearrange("(s p) b -> p s b", p=128), in_=otsb)

    nc.compile()
    return nc


_CACHED = {}


def _get_nc(num_layers=NUM_LAYERS):
    if num_layers not in _CACHED:
        _CACHED[num_layers] = build(num_layers)
    return _CACHED[num_layers]


def make_in_maps(x, w):
    """x [B,C,N] fp32, w [C,D,N] fp32 -> list of 8 per-core input dicts."""
    import ml_dtypes

    bf = ml_dtypes.bfloat16
    x = np.asarray(x, dtype=np.float32)
    w32 = np.ascontiguousarray(np.asarray(w, dtype=np.float32))
    wb = w32.astype(bf)
    wtb = np.ascontiguousarray(w32.transpose(0, 2, 1)).astype(bf)
    bmb = _bmat_np().astype(bf)
    maps = []
    for i in range(N_CORES):
        xs = x[i * BL:(i + 1) * BL]  # [BL, C, N]
        xts = np.ascontiguousarray(xs.transpose(1, 2, 0)).astype(bf)
        maps.append({"xt": xts, "w": wb, "wt": wtb, "bm": bmb})
    return maps


def assemble_out(results):
    outs = []
    for i in range(N_CORES):
        o = results[i]["ot"]  # [C, N, BL]
        outs.append(np.ascontiguousarray(o.transpose(2, 0, 1)))  # [BL, C, N]
    return np.concatenate(outs, axis=0).astype(np.float32)


def kernel(x, W):
    from concourse.bass_utils import run_bass_kernel_spmd

    nc = _get_nc()
    res = run_bass_kernel_spmd(nc, make_in_maps(x, W), list(range(N_CORES)))
    return assemble_out(res.results)


if __name__ == "__main__":
    xs = np.random.randn(B, C, N).astype(np.float32)
    ws = np.random.randn(C, D, N).astype(np.float32)
    ws /= np.linalg.norm(ws, axis=-1, keepdims=True)
    out = kernel(xs, ws)
    print("out", out.shape, out.dtype, float(np.abs(out).mean()))


# revision 8
# speedup vs baseline: 4.9996x; 1.0752x over previous
"""GroupSparseAE (FISTA group-lasso encoder + linear decoder) on 8 trn2 cores.

Data-parallel over batch: each core gets B/8 = 64 rows, W replicated.
Per channel c (3 total, processed sequentially so W[c]/W[c]^T fit in SBUF):
  y2   = TAU * (W @ x^T)                   [D, b] transposed layout
  FISTA iterate k = 1..30 with x in transposed [D, b] layout:
    u^T    = W^T-contract:  uT[n,b]   = sum_d W[d,n] xT[d,b]
    grad^T = gT[e,b]        = sum_n WT[n,e] uT[n,b]
    v      = xT_tmp + y2 - TAU*gT
    group soft-threshold (groups of 8 along d = partition dim):
       gs = Bmat^T @ v^2  (Bmat block-diag ones -> broadcast group sumsq)
       xnew = relu(v) * relu(1 - c/sqrt(gs))
    momentum: xtmp = xnew + m_k (xnew - xold)
  decode: out^T[n,b] = sum_d W[d,n] z[d,b]

Precision: all matmul operands are bf16 (4x faster PE than fp32 which runs
as 2 half-speed LOW/HIGH passes); the FISTA state (xnew/xold/pre/y2) stays
fp32 so quantization error does not accumulate across the 30 iterations
(numpy-simulated rel err 3.7e-3 vs 1.7e-2 with bf16 state).

Engine balance: PSUM accumulator tiles are [128, 512] (one full bank) so
each elementwise op covers 8 matmul d-tiles -> few large DVE/ACT ops
instead of many [128, 64] ones. The u-phase runs t-major so its matmuls
only depend on one 512-wide xtmp chunk at a time and overlap the tail of
the previous activation. dd/tmp elementwise ops go to the idle gpsimd.
"""

import sys

sys.path.insert(0, "/opt/trn_rl_repo")

import numpy as np

B, C, N = 512, 3, 1024
G, S = 256, 8
D = G * S  # 2048
NUM_LAYERS = 30
TAU, LAM = 0.1, 0.1
CTH = LAM * TAU  # group threshold constant

N_CORES = 8
BL = B // N_CORES  # 64 rows per core
NT = D // 128  # 16 d-tiles
NS = N // 128  # 8 n-tiles
FD = NT * BL  # 1024 flat free dim of [D, b] state
CHUNK = 512  # elementwise chunk (8 d-tiles, one PSUM bank)
NCH = FD // CHUNK  # 2
TPC = CHUNK // BL  # 8 d-tiles per chunk

USE_GPSIMD = True  # offload dd/tmp elementwise ops to the idle gpsimd engine


def _mom_coeffs(num_layers):
    # fp32 t-sequence to match the reference's on-device arithmetic
    one, four, two = np.float32(1.0), np.float32(4.0), np.float32(2.0)
    t = np.float32(1.0)
    ms = []
    for _ in range(num_layers):
        t_new = (one + np.sqrt(one + four * t * t)) / two
        ms.append(float((t - one) / t_new))
        t = t_new
    return ms


def _bmat_np():
    p = np.arange(128)
    return (p[:, None] // S == p[None, :] // S).astype(np.float32)


def build(num_layers=NUM_LAYERS):
    import concourse.bacc as bacc
    from concourse import mybir
    from concourse.tile import TileContext

    fp32 = mybir.dt.float32
    bf16 = mybir.dt.bfloat16
    AF = mybir.ActivationFunctionType
    OP = mybir.AluOpType

    nc = bacc.Bacc("TRN2", target_bir_lowering=False, debug=False,
                   num_devices=N_CORES)
    xt = nc.dram_tensor("xt", [C, N, BL], bf16, kind="ExternalInput")
    w = nc.dram_tensor("w", [C, D, N], bf16, kind="ExternalInput")
    wt = nc.dram_tensor("wt", [C, N, D], bf16, kind="ExternalInput")
    bm = nc.dram_tensor("bm", [128, 128], bf16, kind="ExternalInput")
    ot = nc.dram_tensor("ot", [C, N, BL], fp32, kind="ExternalOutput")

    ms = _mom_coeffs(num_layers)

    with TileContext(nc) as tc:
        with (
            tc.tile_pool(name="wp", bufs=2) as wp,
            tc.tile_pool(name="st", bufs=1) as st,
            tc.tile_pool(name="scr", bufs=2) as scr,
            tc.tile_pool(name="ps_u", bufs=2, space="PSUM") as ps_u,
            tc.tile_pool(name="ps_g", bufs=3, space="PSUM") as ps_g,
            tc.tile_pool(name="ps_s", bufs=2, space="PSUM") as ps_s,
        ):
            bmat = wp.tile([128, 128], bf16, tag="bmat")
            nc.sync.dma_start(out=bmat, in_=bm[:, :])

            for c in range(C):
                wsb = wp.tile([128, NT, N], bf16, tag="wsb")
                nc.sync.dma_start(
                    out=wsb, in_=w[c].rearrange("(t p) n -> p t n", p=128))
                wtsb = wp.tile([128, NS, D], bf16, tag="wtsb")
                nc.sync.dma_start(
                    out=wtsb, in_=wt[c].rearrange("(s p) e -> p s e", p=128))
                xts = wp.tile([128, NS, BL], bf16, tag="xts")
                nc.sync.dma_start(
                    out=xts, in_=xt[c].rearrange("(s p) b -> p s b", p=128))

                # persistent per-channel state (fp32 except matmul inputs)
                y2 = st.tile([128, FD], fp32, tag="y2")
                xb0 = st.tile([128, FD], fp32, tag="xb0")
                xb1 = st.tile([128, FD], fp32, tag="xb1")
                xbuf = [xb0, xb1]
                uT = st.tile([128, NS * BL], bf16, tag="uT")
                # chunked tiles for cross-iteration pipelining
                xtmp = [st.tile([128, CHUNK], bf16, tag=f"xtmp{j}",
                                name=f"xtmp{j}") for j in range(NCH)]
                pre = [st.tile([128, CHUNK], fp32, tag=f"pre{j}",
                               name=f"pre{j}") for j in range(NCH)]

                nc.vector.memset(xb0, 0.0)

                # ---- precomp: y2 = TAU * W @ x^T  in [D, b] layout ----
                for j in range(NCH):
                    py = ps_g.tile([128, CHUNK], fp32, tag="pg")
                    for tt in range(TPC):
                        t = j * TPC + tt
                        for s in range(NS):
                            nc.tensor.matmul(
                                py[:, tt * BL:(tt + 1) * BL],
                                wtsb[:, s, t * 128:(t + 1) * 128],
                                xts[:, s, :],
                                start=(tt == 0 and s == 0),
                                stop=(tt == TPC - 1 and s == NS - 1))
                    nc.scalar.mul(y2[:, j * CHUNK:(j + 1) * CHUNK], py, TAU)

                def act_block(vch, k):
                    """vch(j) -> [128, CHUNK] AP of the pre-activation v.
                    Writes xnew (xbuf[k % 2]); unless last iter, also
                    xtmp (bf16 matmul input) and pre (fp32 = xtmp + y2).
                    """
                    xnew, xold = xbuf[k % 2], xbuf[(k - 1) % 2]
                    m = ms[k - 1]
                    last = k == num_layers
                    eng2 = nc.gpsimd if USE_GPSIMD else nc.vector
                    for j in range(NCH):
                        sl = slice(j * CHUNK, (j + 1) * CHUNK)
                        vj = vch(j)
                        v2 = scr.tile([128, CHUNK], bf16, tag="v2")
                        nc.scalar.square(v2, vj)
                        gs = ps_s.tile([128, CHUNK], fp32, tag="gs")
                        nc.tensor.matmul(gs, bmat, v2, start=True, stop=True)
                        nrm = scr.tile([128, CHUNK], fp32, tag="nrm")
                        nc.scalar.sqrt(nrm, gs)
                        invn = scr.tile([128, CHUNK], fp32, tag="invn")
                        nc.vector.reciprocal(invn, nrm)
                        scl = scr.tile([128, CHUNK], fp32, tag="scl")
                        # relu(1 - CTH / nrm)
                        nc.scalar.activation(scl, invn, AF.Relu,
                                             bias=1.0, scale=-CTH)
                        # xnew = max(v, 0) * scl
                        nc.vector.scalar_tensor_tensor(
                            xnew[:, sl], vj, 0.0, scl,
                            op0=OP.max, op1=OP.mult)
                        if not last:
                            dd = scr.tile([128, CHUNK], fp32, tag="dd")
                            eng2.tensor_sub(dd, xnew[:, sl], xold[:, sl])
                            # bf16 matmul input: xtmp = xnew + m*dd
                            nc.vector.scalar_tensor_tensor(
                                xtmp[j], dd, m, xnew[:, sl],
                                op0=OP.mult, op1=OP.add)
                            # fp32 identity path: pre = m*dd + (xnew + y2)
                            tmp = scr.tile([128, CHUNK], fp32, tag="tmp")
                            eng2.tensor_add(tmp, xnew[:, sl], y2[:, sl])
                            nc.vector.scalar_tensor_tensor(
                                pre[j], dd, m, tmp,
                                op0=OP.mult, op1=OP.add)

                # ---- iteration 1: x_tmp = 0 -> v = y2 ----
                act_block(lambda j: y2[:, j * CHUNK:(j + 1) * CHUNK], 1)

                # ---- iterations 2..num_layers ----
                for k in range(2, num_layers + 1):
                    # u-phase (t-major): uT[n,b] = sum_d W[d,n] xtmp[d,b]
                    # 8 accumulation chains (one per n-tile) in one PSUM bank;
                    # matmuls for d-chunk j only wait on xtmp[j].
                    # NOTE first_mm clears has_written for the WHOLE bank, so
                    # only the very first matmul may use start=True: after
                    # that one clear, each chain's first write lands on
                    # cleared bits (-> overwrite) and later ones accumulate.
                    pu = ps_u.tile([128, NS * BL], fp32, tag="pu")
                    for t in range(NT):
                        for s in range(NS):
                            nc.tensor.matmul(
                                pu[:, s * BL:(s + 1) * BL],
                                wsb[:, t, s * 128:(s + 1) * 128],
                                xtmp[t // TPC][:, (t % TPC) * BL:(t % TPC + 1) * BL],
                                start=(t == 0 and s == 0),
                                stop=(t == NT - 1 and s == NS - 1))
                    nc.scalar.copy(uT, pu)
                    # grad-phase chunked: 8 chains (one per d-tile) per bank
                    vt = [scr.tile([128, CHUNK], fp32, tag=f"v{j}",
                                   name=f"v{j}") for j in range(NCH)]
                    for j in range(NCH):
                        pg = ps_g.tile([128, CHUNK], fp32, tag="pg")
                        for tt in range(TPC):
                            t = j * TPC + tt
                            for s in range(NS):
                                nc.tensor.matmul(
                                    pg[:, tt * BL:(tt + 1) * BL],
                                    wtsb[:, s, t * 128:(t + 1) * 128],
                                    uT[:, s * BL:(s + 1) * BL],
                                    start=(tt == 0 and s == 0),
                                    stop=(tt == TPC - 1 and s == NS - 1))
                        # v = pre - TAU * grad
                        nc.vector.scalar_tensor_tensor(
                            vt[j], pg, -TAU, pre[j],
                            op0=OP.mult, op1=OP.add)
                    act_block(lambda j: vt[j][:, :], k)

                # ---- decode: out^T[n,b] = sum_d W[d,n] z[d,b] ----
                z = xbuf[num_layers % 2]
                zbf = st.tile([128, FD], bf16, tag="zbf")
                nc.scalar.copy(zbf, z)
                otsb = st.tile([128, NS, BL], fp32, tag="otsb")
                pd = ps_u.tile([128, NS * BL], fp32, tag="pu")
                for t in range(NT):
                    for s in range(NS):
                        nc.tensor.matmul(
                            pd[:, s * BL:(s + 1) * BL],
                            wsb[:, t, s * 128:(s + 1) * 128],
                            zbf[:, t * BL:(t + 1) * BL],
                            start=(t == 0 and s == 0),
                            stop=(t == NT - 1 and s == NS - 1))
                for s in range(NS):
                    nc.scalar.copy(otsb[:, s, :], pd[:, s * BL:(s + 1) * BL])
                nc.sync.dma_start(
                    out=ot[c].rearrange("(s p) b -> p s b", p=128), in_=otsb)

    nc.compile()
    return nc


_CACHED = {}


def _get_nc(num_layers=NUM_LAYERS):
    if num_layers not in _CACHED:
        _CACHED[num_layers] = build(num_layers)
    return _CACHED[num_layers]


def make_in_maps(x, w):
    """x [B,C,N] fp32, w [C,D,N] fp32 -> list of 8 per-core input dicts."""
    import ml_dtypes

    bf = ml_dtypes.bfloat16
    x = np.asarray(x, dtype=np.float32)
    w32 = np.ascontiguousarray(np.asarray(w, dtype=np.float32))
    wb = w32.astype(bf)
    wtb = np.ascontiguousarray(w32.transpose(0, 2, 1)).astype(bf)
    bmb = _bmat_np().astype(bf)
    maps = []
    for i in range(N_CORES):
        xs = x[i * BL:(i + 1) * BL]  # [BL, C, N]
        xts = np.ascontiguousarray(xs.transpose(1, 2, 0)).astype(bf)
        maps.append({"xt": xts, "w": wb, "wt": wtb, "bm": bmb})
    return maps


def assemble_out(results):
    outs = []
    for i in range(N_CORES):
        o = results[i]["ot"]  # [C, N, BL]
        outs.append(np.ascontiguousarray(o.transpose(2, 0, 1)))  # [BL, C, N]
    return np.concatenate(outs, axis=0).astype(np.float32)


def kernel(x, W):
    from concourse.bass_utils import run_bass_kernel_spmd

    nc = _get_nc()
    res = run_bass_kernel_spmd(nc, make_in_maps(x, W), list(range(N_CORES)))
    return assemble_out(res.results)


if __name__ == "__main__":
    xs = np.random.randn(B, C, N).astype(np.float32)
    ws = np.random.randn(C, D, N).astype(np.float32)
    ws /= np.linalg.norm(ws, axis=-1, keepdims=True)
    out = kernel(xs, ws)
    print("out", out.shape, out.dtype, float(np.abs(out).mean()))


# revision 12
# speedup vs baseline: 6.6892x; 1.3379x over previous
"""GroupSparseAE (FISTA group-lasso encoder + linear decoder) on 8 trn2 cores.

Data-parallel over batch: each core gets B/8 = 64 rows, W replicated.
Per channel c (3 total, processed sequentially so W[c]/W[c]^T fit in SBUF):
  y2   = TAU * (W @ x^T)                   [D, b] transposed layout
  FISTA iterate k = 1..30 with x in transposed [D, b] layout:
    u^T    = W^T-contract:  uT[n,b]   = sum_d W[d,n] xT[d,b]
    grad^T = gT[e,b]        = sum_n WT[n,e] uT[n,b]
    v      = xT_tmp + y2 - TAU*gT
    group soft-threshold (groups of 8 along d = partition dim):
       gs = Bmat^T @ v^2  (Bmat block-diag ones -> broadcast group sumsq)
       xnew = relu(v) * relu(1 - c/sqrt(gs))
    momentum: xtmp = xnew + m_k (xnew - xold)
  decode: out^T[n,b] = sum_d W[d,n] z[d,b]

Precision: all matmul operands are bf16 (4x faster PE than fp32 which runs
as 2 half-speed LOW/HIGH passes); the FISTA state (xnew/xold/pre/y2) stays
fp32 so quantization error does not accumulate across the 30 iterations
(numpy-simulated rel err 3.7e-3 vs 1.7e-2 with bf16 state).

Engine balance: PSUM accumulator tiles are [128, 512] (one full bank) so
each elementwise op covers 8 matmul d-tiles -> few large DVE/ACT ops
instead of many [128, 64] ones. The u-phase runs t-major so its matmuls
only depend on one 512-wide xtmp chunk at a time and overlap the tail of
the previous activation. dd/tmp elementwise ops go to the idle gpsimd.
"""

import sys

sys.path.insert(0, "/opt/trn_rl_repo")

import numpy as np

B, C, N = 512, 3, 1024
G, S = 256, 8
D = G * S  # 2048
NUM_LAYERS = 30
TAU, LAM = 0.1, 0.1
CTH = LAM * TAU  # group threshold constant

N_CORES = 8
BL = B // N_CORES  # 64 rows per core
NT = D // 128  # 16 d-tiles
NS = N // 128  # 8 n-tiles
FD = NT * BL  # 1024 flat free dim of [D, b] state
CHUNK = 512  # elementwise chunk (8 d-tiles, one PSUM bank)
NCH = FD // CHUNK  # 2
TPC = CHUNK // BL  # 8 d-tiles per chunk

USE_GPSIMD = True  # offload dd/tmp elementwise ops to the idle gpsimd engine


def _mom_coeffs(num_layers):
    # fp32 t-sequence to match the reference's on-device arithmetic
    one, four, two = np.float32(1.0), np.float32(4.0), np.float32(2.0)
    t = np.float32(1.0)
    ms = []
    for _ in range(num_layers):
        t_new = (one + np.sqrt(one + four * t * t)) / two
        ms.append(float((t - one) / t_new))
        t = t_new
    return ms


def _bmat_np():
    p = np.arange(128)
    return (p[:, None] // S == p[None, :] // S).astype(np.float32)


def build(num_layers=NUM_LAYERS):
    import concourse.bacc as bacc
    from concourse import mybir
    from concourse.tile import TileContext

    fp32 = mybir.dt.float32
    bf16 = mybir.dt.bfloat16
    AF = mybir.ActivationFunctionType
    OP = mybir.AluOpType

    nc = bacc.Bacc("TRN2", target_bir_lowering=False, debug=False,
                   num_devices=N_CORES)
    xt = nc.dram_tensor("xt", [C, N, BL], bf16, kind="ExternalInput")
    w = nc.dram_tensor("w", [C, D, N], bf16, kind="ExternalInput")
    wt = nc.dram_tensor("wt", [C, N, D], bf16, kind="ExternalInput")
    bm = nc.dram_tensor("bm", [128, 128], bf16, kind="ExternalInput")
    ot = nc.dram_tensor("ot", [C, N, BL], fp32, kind="ExternalOutput")

    ms = _mom_coeffs(num_layers)

    with TileContext(nc) as tc:
        with (
            tc.tile_pool(name="wp", bufs=2) as wp,
            tc.tile_pool(name="st", bufs=1) as st,
            tc.tile_pool(name="scr", bufs=2) as scr,
            tc.tile_pool(name="ps_u", bufs=2, space="PSUM") as ps_u,
            tc.tile_pool(name="ps_g", bufs=3, space="PSUM") as ps_g,
            tc.tile_pool(name="ps_s", bufs=2, space="PSUM") as ps_s,
        ):
            bmat = wp.tile([128, 128], bf16, tag="bmat")
            nc.sync.dma_start(out=bmat, in_=bm[:, :])
            eps = wp.tile([128, 1], fp32, tag="eps")
            nc.vector.memset(eps, 1e-30)

            for c in range(C):
                wsb = wp.tile([128, NT, N], bf16, tag="wsb")
                nc.sync.dma_start(
                    out=wsb, in_=w[c].rearrange("(t p) n -> p t n", p=128))
                wtsb = wp.tile([128, NS, D], bf16, tag="wtsb")
                nc.sync.dma_start(
                    out=wtsb, in_=wt[c].rearrange("(s p) e -> p s e", p=128))
                xts = wp.tile([128, NS, BL], bf16, tag="xts")
                nc.sync.dma_start(
                    out=xts, in_=xt[c].rearrange("(s p) b -> p s b", p=128))

                # persistent per-channel state (fp32 except matmul inputs)
                y2 = st.tile([128, FD], fp32, tag="y2")
                xb0 = st.tile([128, FD], fp32, tag="xb0")
                xb1 = st.tile([128, FD], fp32, tag="xb1")
                xbuf = [xb0, xb1]
                uT = st.tile([128, NS * BL], bf16, tag="uT")
                # chunked tiles for cross-iteration pipelining
                xtmp = [st.tile([128, CHUNK], bf16, tag=f"xtmp{j}",
                                name=f"xtmp{j}") for j in range(NCH)]
                pre = [st.tile([128, CHUNK], fp32, tag=f"pre{j}",
                               name=f"pre{j}") for j in range(NCH)]

                nc.vector.memset(xb0, 0.0)

                # ---- precomp: y2 = TAU * W @ x^T  in [D, b] layout ----
                for j in range(NCH):
                    py = ps_g.tile([128, CHUNK], fp32, tag="pg")
                    for tt in range(TPC):
                        t = j * TPC + tt
                        for s in range(NS):
                            nc.tensor.matmul(
                                py[:, tt * BL:(tt + 1) * BL],
                                wtsb[:, s, t * 128:(t + 1) * 128],
                                xts[:, s, :],
                                start=(tt == 0 and s == 0),
                                stop=(tt == TPC - 1 and s == NS - 1))
                    nc.scalar.mul(y2[:, j * CHUNK:(j + 1) * CHUNK], py, TAU)

                def act_block(vch, k):
                    """vch(j) -> [128, CHUNK] AP of the pre-activation v.
                    Writes xnew (xbuf[k % 2]); unless last iter, also
                    xtmp (bf16 matmul input) and pre (fp32 = xtmp + y2).
                    """
                    xnew, xold = xbuf[k % 2], xbuf[(k - 1) % 2]
                    m = ms[k - 1]
                    last = k == num_layers
                    eng2 = nc.gpsimd if USE_GPSIMD else nc.vector
                    for j in range(NCH):
                        sl = slice(j * CHUNK, (j + 1) * CHUNK)
                        vj = vch(j)
                        v2 = scr.tile([128, CHUNK], bf16, tag="v2")
                        nc.scalar.square(v2, vj)
                        gs = ps_s.tile([128, CHUNK], fp32, tag="gs")
                        nc.tensor.matmul(gs, bmat, v2, start=True, stop=True)
                        nrm = scr.tile([128, CHUNK], fp32, tag="nrm")
                        # +1e-30 guards reciprocal_approx_fast against the
                        # undefined 0-input case; for nrm ~ 1e-15 the scale
                        # relu(1 - CTH/nrm) is exactly 0, matching reference.
                        nc.scalar.activation(nrm, gs, AF.Sqrt,
                                             bias=eps[:, :], scale=1.0)
                        invn = scr.tile([128, CHUNK], fp32, tag="invn")
                        nc.vector.reciprocal_approx_fast(invn, nrm)
                        scl = scr.tile([128, CHUNK], fp32, tag="scl")
                        # relu(1 - CTH / nrm)
                        nc.scalar.activation(scl, invn, AF.Relu,
                                             bias=1.0, scale=-CTH)
                        # xnew = max(v, 0) * scl
                        nc.vector.scalar_tensor_tensor(
                            xnew[:, sl], vj, 0.0, scl,
                            op0=OP.max, op1=OP.mult)
                        if not last:
                            # dd is on the critical chain that gates the next
                            # iteration's u-phase matmuls -> keep it on DVE
                            dd = scr.tile([128, CHUNK], fp32, tag="dd")
                            nc.vector.tensor_sub(dd, xnew[:, sl], xold[:, sl])
                            # bf16 matmul input: xtmp = xnew + m*dd
                            nc.vector.scalar_tensor_tensor(
                                xtmp[j], dd, m, xnew[:, sl],
                                op0=OP.mult, op1=OP.add)
                            # fp32 identity path: pre = m*dd + (xnew + y2)
                            tmp = scr.tile([128, CHUNK], fp32, tag="tmp")
                            eng2.tensor_add(tmp, xnew[:, sl], y2[:, sl])
                            nc.vector.scalar_tensor_tensor(
                                pre[j], dd, m, tmp,
                                op0=OP.mult, op1=OP.add)

                # ---- iteration 1: x_tmp = 0 -> v = y2 ----
                act_block(lambda j: y2[:, j * CHUNK:(j + 1) * CHUNK], 1)

                # ---- iterations 2..num_layers ----
                for k in range(2, num_layers + 1):
                    # u-phase (t-major): uT[n,b] = sum_d W[d,n] xtmp[d,b]
                    # 8 accumulation chains (one per n-tile) in one PSUM bank;
                    # matmuls for d-chunk j only wait on xtmp[j].
                    # NOTE first_mm clears has_written for the WHOLE bank, so
                    # only the very first matmul may use start=True: after
                    # that one clear, each chain's first write lands on
                    # cleared bits (-> overwrite) and later ones accumulate.
                    pu = ps_u.tile([128, NS * BL], fp32, tag="pu")
                    for t in range(NT):
                        for s in range(NS):
                            nc.tensor.matmul(
                                pu[:, s * BL:(s + 1) * BL],
                                wsb[:, t, s * 128:(s + 1) * 128],
                                xtmp[t // TPC][:, (t % TPC) * BL:(t % TPC + 1) * BL],
                                start=(t == 0 and s == 0),
                                stop=(t == NT - 1 and s == NS - 1))
                    nc.scalar.copy(uT, pu)
                    # grad-phase chunked: 8 chains (one per d-tile) per bank
                    vt = [scr.tile([128, CHUNK], fp32, tag=f"v{j}",
                                   name=f"v{j}") for j in range(NCH)]
                    for j in range(NCH):
                        pg = ps_g.tile([128, CHUNK], fp32, tag="pg")
                        for tt in range(TPC):
                            t = j * TPC + tt
                            for s in range(NS):
                                nc.tensor.matmul(
                                    pg[:, tt * BL:(tt + 1) * BL],
                                    wtsb[:, s, t * 128:(t + 1) * 128],
                                    uT[:, s * BL:(s + 1) * BL],
                                    start=(tt == 0 and s == 0),
                                    stop=(tt == TPC - 1 and s == NS - 1))
                        # v = pre - TAU * grad
                        nc.vector.scalar_tensor_tensor(
                            vt[j], pg, -TAU, pre[j],
                            op0=OP.mult, op1=OP.add)
                    act_block(lambda j: vt[j][:, :], k)

                # ---- decode: out^T[n,b] = sum_d W[d,n] z[d,b] ----
                z = xbuf[num_layers % 2]
                zbf = st.tile([128, FD], bf16, tag="zbf")
                nc.scalar.copy(zbf, z)
                otsb = st.tile([128, NS, BL], fp32, tag="otsb")
                pd = ps_u.tile([128, NS * BL], fp32, tag="pu")
                for t in range(NT):
                    for s in range(NS):
                        nc.tensor.matmul(
                            pd[:, s * BL:(s + 1) * BL],
                            wsb[:, t, s * 128:(s + 1) * 128],
                            zbf[:, t * BL:(t + 1) * BL],
                            start=(t == 0 and s == 0),
                            stop=(t == NT - 1 and s == NS - 1))
                for s in range(NS):
                    nc.scalar.copy(otsb[:, s, :], pd[:, s * BL:(s + 1) * BL])
                nc.sync.dma_start(
                    out=ot[c].rearrange("(s p) b -> p s b", p=128), in_=otsb)

    nc.compile()
    return nc


_CACHED = {}


def _get_nc(num_layers=NUM_LAYERS):
    if num_layers not in _CACHED:
        _CACHED[num_layers] = build(num_layers)
    return _CACHED[num_layers]


def make_in_maps(x, w):
    """x [B,C,N] fp32, w [C,D,N] fp32 -> list of 8 per-core input dicts."""
    import ml_dtypes

    bf = ml_dtypes.bfloat16
    x = np.asarray(x, dtype=np.float32)
    w32 = np.ascontiguousarray(np.asarray(w, dtype=np.float32))
    wb = w32.astype(bf)
    wtb = np.ascontiguousarray(w32.transpose(0, 2, 1)).astype(bf)
    bmb = _bmat_np().astype(bf)
    maps = []
    for i in range(N_CORES):
        xs = x[i * BL:(i + 1) * BL]  # [BL, C, N]
        xts = np.ascontiguousarray(xs.transpose(1, 2, 0)).astype(bf)
        maps.append({"xt": xts, "w": wb, "wt": wtb, "bm": bmb})
    return maps


def assemble_out(results):
    outs = []
    for i in range(N_CORES):
        o = results[i]["ot"]  # [C, N, BL]
        outs.append(np.ascontiguousarray(o.transpose(2, 0, 1)))  # [BL, C, N]
    return np.concatenate(outs, axis=0).astype(np.float32)


def kernel(x, W):
    from concourse.bass_utils import run_bass_kernel_spmd

    nc = _get_nc()
    res = run_bass_kernel_spmd(nc, make_in_maps(x, W), list(range(N_CORES)))
    return assemble_out(res.results)


if __name__ == "__main__":
    xs = np.random.randn(B, C, N).astype(np.float32)
    ws = np.random.randn(C, D, N).astype(np.float32)
    ws /= np.linalg.norm(ws, axis=-1, keepdims=True)
    out = kernel(xs, ws)
    print("out", out.shape, out.dtype, float(np.abs(out).mean()))


# revision 17
# speedup vs baseline: 9.4904x; 1.4188x over previous
"""GroupSparseAE (FISTA group-lasso encoder + linear decoder) on 8 trn2 cores.

Sharding (channel-major, 1536 (row, channel) atoms over 8 cores):
  sub A: cores 0-3 take channel 0 rows [128i, 128i+128); cores 4-7 take
         channel 1 rows likewise  -> 128-row block, one W.
  sub B: every core takes channel 2 rows [64i, 64i+64) -> 64-row block.
A's 128-wide moving operand halves the LDWEIGHTS-per-row cost vs 64-wide.

A and B are independent FISTA problems; their per-iteration pipelines are
interleaved (A-matmuls, B-matmuls, A-activation, B-activation) so each
sub's serial activation chain (square -> group-sum matmul -> sqrt -> recip
-> scale -> threshold -> momentum) hides under the other sub's matmuls.

Per sub, per iteration k = 1..30 with state in transposed [D, b] layout:
    u^T    = W^T-contract:  uT[n,b]   = sum_d W[d,n] xT[d,b]
    grad^T = gT[e,b]        = sum_n WT[n,e] uT[n,b]
    v      = xT_tmp + y2 - TAU*gT          (y2 = TAU * W @ x^T, precomputed)
    group soft-threshold (groups of 8 along d = partition dim):
       gs = Bmat^T @ v^2  (Bmat block-diag ones -> broadcast group sumsq)
       xnew = relu(v) * relu(1 - c/sqrt(gs))
    momentum: xtmp = xnew + m_k (xnew - xold)
  decode: out^T[n,b] = sum_d W[d,n] z[d,b]

Precision: all matmul operands bf16 (fp32 matmul = 2 half-speed passes);
the FISTA state (xnew/xold/pre) stays fp32 so quantization does not
accumulate over 30 iterations (numpy sim: 4.6e-3 rel err vs 1.7e-2 with
bf16 state). y2 is stored bf16 (constant perturbation only).

PSUM accumulators are [128, 512] (one bank, 8/4 interleaved chains); only
the first matmul into a bank uses start=True, because first_mm clears the
has_written bits of the WHOLE bank.
"""

import sys

sys.path.insert(0, "/opt/trn_rl_repo")

import numpy as np

B, C, N = 512, 3, 1024
G, S = 256, 8
D = G * S  # 2048
NUM_LAYERS = 30
TAU, LAM = 0.1, 0.1
CTH = LAM * TAU  # group threshold constant

N_CORES = 8
NT = D // 128  # 16 d-tiles
NS = N // 128  # 8 n-tiles
BLA = 128  # sub-A rows per core
BLB = 64  # sub-B rows per core
CHUNK = 512  # elementwise chunk, one PSUM bank of fp32


def _mom_coeffs(num_layers):
    # fp32 t-sequence to match the reference's on-device arithmetic
    one, four, two = np.float32(1.0), np.float32(4.0), np.float32(2.0)
    t = np.float32(1.0)
    ms = []
    for _ in range(num_layers):
        t_new = (one + np.sqrt(one + four * t * t)) / two
        ms.append(float((t - one) / t_new))
        t = t_new
    return ms


def _bmat_np():
    p = np.arange(128)
    return (p[:, None] // S == p[None, :] // S).astype(np.float32)


def build(num_layers=NUM_LAYERS):
    import concourse.bacc as bacc
    from concourse import mybir
    from concourse.tile import TileContext

    fp32 = mybir.dt.float32
    bf16 = mybir.dt.bfloat16
    AF = mybir.ActivationFunctionType
    OP = mybir.AluOpType

    nc = bacc.Bacc("TRN2", target_bir_lowering=False, debug=False,
                   num_devices=N_CORES)
    xta = nc.dram_tensor("xta", [N, BLA], bf16, kind="ExternalInput")
    wa = nc.dram_tensor("wa", [D, N], bf16, kind="ExternalInput")
    wta = nc.dram_tensor("wta", [N, D], bf16, kind="ExternalInput")
    xtb = nc.dram_tensor("xtb", [N, BLB], bf16, kind="ExternalInput")
    wb = nc.dram_tensor("wb", [D, N], bf16, kind="ExternalInput")
    wtb = nc.dram_tensor("wtb", [N, D], bf16, kind="ExternalInput")
    bm = nc.dram_tensor("bm", [128, 128], bf16, kind="ExternalInput")
    ota = nc.dram_tensor("ota", [N, BLA], fp32, kind="ExternalOutput")
    otb = nc.dram_tensor("otb", [N, BLB], fp32, kind="ExternalOutput")

    ms = _mom_coeffs(num_layers)

    with TileContext(nc) as tc:
        with (
            tc.tile_pool(name="wp", bufs=1) as wp,
            tc.tile_pool(name="st", bufs=1) as st,
            tc.tile_pool(name="scr", bufs=2) as scr,
            tc.tile_pool(name="scr1", bufs=1) as scr1,
            tc.tile_pool(name="ps_u", bufs=3, space="PSUM") as ps_u,
            tc.tile_pool(name="ps_g", bufs=3, space="PSUM") as ps_g,
            tc.tile_pool(name="ps_s", bufs=2, space="PSUM") as ps_s,
        ):
            bmat = wp.tile([128, 128], bf16, tag="bmat")
            eps = wp.tile([128, 1], fp32, tag="eps")
            nc.vector.memset(eps, 1e-30)

            class Sub:
                def __init__(self, q, bl, xt_d, w_d, wt_d, ot_d):
                    self.q = q
                    self.bl = bl
                    self.fd = NT * bl
                    self.nch = self.fd // CHUNK
                    self.tpc = CHUNK // bl  # d-tiles per chunk
                    self.sgp = CHUNK // bl  # s-tiles per pu bank
                    self.npu = NS // self.sgp
                    self.ot_d = ot_d
                    self.wsb = wp.tile([128, NT, N], bf16, tag=f"wsb{q}", name=f"wsb{q}")
                    self.wtsb = wp.tile([128, NS, D], bf16, tag=f"wtsb{q}", name=f"wtsb{q}")
                    self.xts = wp.tile([128, NS, bl], bf16, tag=f"xts{q}", name=f"xts{q}")
                    self.dma_w = lambda: nc.sync.dma_start(
                        out=self.wsb, in_=w_d.rearrange("(t p) n -> p t n", p=128))
                    self.dma_wt = lambda: nc.sync.dma_start(
                        out=self.wtsb, in_=wt_d.rearrange("(s p) e -> p s e", p=128))
                    self.dma_x = lambda: nc.sync.dma_start(
                        out=self.xts, in_=xt_d.rearrange("(s p) b -> p s b", p=128))
                    self.y2 = st.tile([128, self.fd], bf16, tag=f"y2{q}", name=f"y2{q}")
                    self.xb = [st.tile([128, self.fd], fp32, tag=f"xb{i}{q}",
                                       name=f"xb{i}{q}") for i in range(2)]
                    self.uT = st.tile([128, NS * bl], bf16, tag=f"uT{q}", name=f"uT{q}")
                    self.xtmp = [st.tile([128, CHUNK], bf16, tag=f"xtmp{j}{q}",
                                         name=f"xtmp{j}{q}") for j in range(self.nch)]
                    self.pre = [st.tile([128, CHUNK], fp32, tag=f"pre{j}{q}",
                                        name=f"pre{j}{q}") for j in range(self.nch)]
                    nc.vector.memset(self.xb[0], 0.0)

                def y2_phase(self):
                    bl, tpc = self.bl, self.tpc
                    for j in range(self.nch):
                        py = ps_g.tile([128, CHUNK], fp32, tag="pg")
                        for tt in range(tpc):
                            t = j * tpc + tt
                            for s in range(NS):
                                nc.tensor.matmul(
                                    py[:, tt * bl:(tt + 1) * bl],
                                    self.wtsb[:, s, t * 128:(t + 1) * 128],
                                    self.xts[:, s, :],
                                    start=(tt == 0 and s == 0),
                                    stop=(tt == tpc - 1 and s == NS - 1))
                        nc.scalar.mul(
                            self.y2[:, j * CHUNK:(j + 1) * CHUNK], py, TAU)

                def mm_phase(self, k):
                    """u-phase + grad-phase + v-combine (v written in-place
                    into pre). Emitted t-major so u matmuls for d-chunk j
                    only wait on xtmp[j]."""
                    bl, tpc, sgp = self.bl, self.tpc, self.sgp
                    pus = [ps_u.tile([128, CHUNK], fp32, tag="pu",
                                     name=f"pu{self.q}{p}") for p in range(self.npu)]
                    for t in range(NT):
                        for s in range(NS):
                            nc.tensor.matmul(
                                pus[s // sgp][:, (s % sgp) * bl:(s % sgp + 1) * bl],
                                self.wsb[:, t, s * 128:(s + 1) * 128],
                                self.xtmp[t // tpc][:, (t % tpc) * bl:(t % tpc + 1) * bl],
                                start=(t == 0 and s % sgp == 0),
                                stop=(t == NT - 1 and s % sgp == sgp - 1))
                    for p in range(self.npu):
                        nc.scalar.copy(
                            self.uT[:, p * CHUNK:(p + 1) * CHUNK], pus[p])
                    for j in range(self.nch):
                        pg = ps_g.tile([128, CHUNK], fp32, tag="pg")
                        for tt in range(tpc):
                            t = j * tpc + tt
                            for s in range(NS):
                                nc.tensor.matmul(
                                    pg[:, tt * bl:(tt + 1) * bl],
                                    self.wtsb[:, s, t * 128:(t + 1) * 128],
                                    self.uT[:, s * bl:(s + 1) * bl],
                                    start=(tt == 0 and s == 0),
                                    stop=(tt == tpc - 1 and s == NS - 1))
                        # v = pre - TAU*grad, in place (pre is rebuilt below)
                        nc.vector.scalar_tensor_tensor(
                            self.pre[j], pg, -TAU, self.pre[j],
                            op0=OP.mult, op1=OP.add)

                def act_phase(self, k):
                    """Group soft-threshold + momentum on each chunk.
                    k == 1 reads v from y2; else v is in pre (in-place)."""
                    xnew, xold = self.xb[k % 2], self.xb[(k - 1) % 2]
                    m = ms[k - 1]
                    last = k == num_layers
                    for j in range(self.nch):
                        sl = slice(j * CHUNK, (j + 1) * CHUNK)
                        vj = self.y2[:, sl] if k == 1 else self.pre[j][:, :]
                        v2 = scr1.tile([128, CHUNK], bf16, tag="v2")
                        nc.scalar.square(v2, vj)
                        gs = ps_s.tile([128, CHUNK], fp32, tag="gs")
                        nc.tensor.matmul(gs, bmat, v2, start=True, stop=True)
                        nrm = scr.tile([128, CHUNK], fp32, tag="nrm")
                        # +1e-30 guards reciprocal_approx_fast's undefined
                        # 0-input; relu(1 - CTH/1e-15) = 0 matches reference
                        nc.scalar.activation(nrm, gs, AF.Sqrt,
                                             bias=eps[:, :], scale=1.0)
                        invn = scr.tile([128, CHUNK], fp32, tag="invn")
                        nc.vector.reciprocal_approx_fast(invn, nrm)
                        scl = scr.tile([128, CHUNK], fp32, tag="scl")
                        nc.scalar.activation(scl, invn, AF.Relu,
                                             bias=1.0, scale=-CTH)
                        # xnew = max(v, 0) * scl
                        nc.vector.scalar_tensor_tensor(
                            xnew[:, sl], vj, 0.0, scl,
                            op0=OP.max, op1=OP.mult)
                        if not last:
                            dd = scr.tile([128, CHUNK], fp32, tag="dd")
                            nc.gpsimd.tensor_sub(dd, xnew[:, sl], xold[:, sl])
                            # bf16 matmul input: xtmp = xnew + m*dd
                            nc.vector.scalar_tensor_tensor(
                                self.xtmp[j], dd, m, xnew[:, sl],
                                op0=OP.mult, op1=OP.add)
                            # fp32 identity path: pre = (m*dd + xnew) + y2
                            nc.vector.scalar_tensor_tensor(
                                self.pre[j], dd, m, xnew[:, sl],
                                op0=OP.mult, op1=OP.add)
                            nc.gpsimd.tensor_add(self.pre[j], self.pre[j],
                                                 self.y2[:, sl])

                def decode(self):
                    bl, sgp = self.bl, self.sgp
                    z = self.xb[num_layers % 2]
                    zbf = st.tile([128, self.fd], bf16, tag="zbf")
                    nc.scalar.copy(zbf, z)
                    pds = [ps_u.tile([128, CHUNK], fp32, tag="pu",
                                     name=f"pd{self.q}{p}") for p in range(self.npu)]
                    for t in range(NT):
                        for s in range(NS):
                            nc.tensor.matmul(
                                pds[s // sgp][:, (s % sgp) * bl:(s % sgp + 1) * bl],
                                self.wsb[:, t, s * 128:(s + 1) * 128],
                                zbf[:, t * bl:(t + 1) * bl],
                                start=(t == 0 and s % sgp == 0),
                                stop=(t == NT - 1 and s % sgp == sgp - 1))
                    otsb = st.tile([128, NS, bl], fp32, tag="otsb")
                    for s in range(NS):
                        nc.scalar.copy(
                            otsb[:, s, :],
                            pds[s // sgp][:, (s % sgp) * bl:(s % sgp + 1) * bl])
                    nc.sync.dma_start(
                        out=self.ot_d.rearrange("(s p) b -> p s b", p=128),
                        in_=otsb)

            A = Sub("a", BLA, xta, wa, wta, ota)
            Bs = Sub("b", BLB, xtb, wb, wtb, otb)
            nc.sync.dma_start(out=bmat, in_=bm[:, :])
            A.dma_wt(); A.dma_x()
            Bs.dma_wt(); Bs.dma_x()
            A.dma_w(); Bs.dma_w()

            A.y2_phase(); Bs.y2_phase()
            A.act_phase(1); Bs.act_phase(1)
            for k in range(2, num_layers + 1):
                A.mm_phase(k)
                Bs.mm_phase(k)
                A.act_phase(k)
                Bs.act_phase(k)
            A.decode(); Bs.decode()

    nc.compile()
    return nc


_CACHED = {}


def _get_nc(num_layers=NUM_LAYERS):
    if num_layers not in _CACHED:
        _CACHED[num_layers] = build(num_layers)
    return _CACHED[num_layers]


def make_in_maps(x, w):
    """x [B,C,N] fp32, w [C,D,N] fp32 -> list of 8 per-core input dicts."""
    import ml_dtypes

    bf = ml_dtypes.bfloat16
    x = np.asarray(x, dtype=np.float32)
    w32 = np.ascontiguousarray(np.asarray(w, dtype=np.float32))
    wb_ = w32.astype(bf)
    wtb_ = np.ascontiguousarray(w32.transpose(0, 2, 1)).astype(bf)
    bmb = _bmat_np().astype(bf)
    maps = []
    for i in range(N_CORES):
        ca = 0 if i < 4 else 1
        ra = (i % 4) * BLA
        xa = np.ascontiguousarray(x[ra:ra + BLA, ca].T).astype(bf)  # [N, BLA]
        rb = i * BLB
        xb_ = np.ascontiguousarray(x[rb:rb + BLB, 2].T).astype(bf)  # [N, BLB]
        maps.append({
            "xta": xa, "wa": wb_[ca], "wta": wtb_[ca],
            "xtb": xb_, "wb": wb_[2], "wtb": wtb_[2], "bm": bmb,
        })
    return maps


def assemble_out(results):
    out = np.empty((B, C, N), np.float32)
    for i in range(N_CORES):
        ca = 0 if i < 4 else 1
        ra = (i % 4) * BLA
        out[ra:ra + BLA, ca] = results[i]["ota"].T  # [N, BLA] -> [BLA, N]
        rb = i * BLB
        out[rb:rb + BLB, 2] = results[i]["otb"].T
    return out


def kernel(x, W):
    from concourse.bass_utils import run_bass_kernel_spmd

    nc = _get_nc()
    res = run_bass_kernel_spmd(nc, make_in_maps(x, W), list(range(N_CORES)))
    return assemble_out(res.results)


if __name__ == "__main__":
    xs = np.random.randn(B, C, N).astype(np.float32)
    ws = np.random.randn(C, D, N).astype(np.float32)
    ws /= np.linalg.norm(ws, axis=-1, keepdims=True)
    out = kernel(xs, ws)
    print("out", out.shape, out.dtype, float(np.abs(out).mean()))


# revision 19
# speedup vs baseline: 9.4985x; 1.0009x over previous
"""GroupSparseAE (FISTA group-lasso encoder + linear decoder) on 8 trn2 cores.

Sharding (channel-major, 1536 (row, channel) atoms over 8 cores):
  sub A: cores 0-3 take channel 0 rows [128i, 128i+128); cores 4-7 take
         channel 1 rows likewise  -> 128-row block, one W.
  sub B: every core takes channel 2 rows [64i, 64i+64) -> 64-row block.
A's 128-wide moving operand halves the LDWEIGHTS-per-row cost vs 64-wide.

A and B are independent FISTA problems; their per-iteration pipelines are
interleaved (A-matmuls, B-matmuls, A-activation, B-activation) so each
sub's serial activation chain (square -> group-sum matmul -> sqrt -> recip
-> scale -> threshold -> momentum) hides under the other sub's matmuls.

Per sub, per iteration k = 1..30 with state in transposed [D, b] layout:
    u^T    = W^T-contract:  uT[n,b]   = sum_d W[d,n] xT[d,b]
    grad^T = gT[e,b]        = sum_n WT[n,e] uT[n,b]
    v      = xT_tmp + y2 - TAU*gT          (y2 = TAU * W @ x^T, precomputed)
    group soft-threshold (groups of 8 along d = partition dim):
       gs = Bmat^T @ v^2  (Bmat block-diag ones -> broadcast group sumsq)
       xnew = relu(v) * relu(1 - c/sqrt(gs))
    momentum: xtmp = xnew + m_k (xnew - xold)
  decode: out^T[n,b] = sum_d W[d,n] z[d,b]

Precision: all matmul operands bf16 (fp32 matmul = 2 half-speed passes);
the FISTA state (xnew/xold/pre) stays fp32 so quantization does not
accumulate over 30 iterations (numpy sim: 4.6e-3 rel err vs 1.7e-2 with
bf16 state). y2 is stored bf16 (constant perturbation only).

PSUM accumulators are [128, 512] (one bank, 8/4 interleaved chains); only
the first matmul into a bank uses start=True, because first_mm clears the
has_written bits of the WHOLE bank.
"""

import sys

sys.path.insert(0, "/opt/trn_rl_repo")

import numpy as np

B, C, N = 512, 3, 1024
G, S = 256, 8
D = G * S  # 2048
NUM_LAYERS = 30
TAU, LAM = 0.1, 0.1
CTH = LAM * TAU  # group threshold constant

N_CORES = 8
NT = D // 128  # 16 d-tiles
NS = N // 128  # 8 n-tiles
BLA = 128  # sub-A rows per core
BLB = 64  # sub-B rows per core
CHUNK = 512  # elementwise chunk, one PSUM bank of fp32


def _mom_coeffs(num_layers):
    # fp32 t-sequence to match the reference's on-device arithmetic
    one, four, two = np.float32(1.0), np.float32(4.0), np.float32(2.0)
    t = np.float32(1.0)
    ms = []
    for _ in range(num_layers):
        t_new = (one + np.sqrt(one + four * t * t)) / two
        ms.append(float((t - one) / t_new))
        t = t_new
    return ms


def _bmat_np():
    p = np.arange(128)
    return (p[:, None] // S == p[None, :] // S).astype(np.float32)


def build(num_layers=NUM_LAYERS):
    import concourse.bacc as bacc
    from concourse import mybir
    from concourse.tile import TileContext

    fp32 = mybir.dt.float32
    bf16 = mybir.dt.bfloat16
    AF = mybir.ActivationFunctionType
    OP = mybir.AluOpType

    nc = bacc.Bacc("TRN2", target_bir_lowering=False, debug=False,
                   num_devices=N_CORES)
    xta = nc.dram_tensor("xta", [N, BLA], bf16, kind="ExternalInput")
    wa = nc.dram_tensor("wa", [D, N], bf16, kind="ExternalInput")
    wta = nc.dram_tensor("wta", [N, D], bf16, kind="ExternalInput")
    xtb = nc.dram_tensor("xtb", [N, BLB], bf16, kind="ExternalInput")
    wb = nc.dram_tensor("wb", [D, N], bf16, kind="ExternalInput")
    wtb = nc.dram_tensor("wtb", [N, D], bf16, kind="ExternalInput")
    bm = nc.dram_tensor("bm", [128, 128], bf16, kind="ExternalInput")
    ota = nc.dram_tensor("ota", [N, BLA], fp32, kind="ExternalOutput")
    otb = nc.dram_tensor("otb", [N, BLB], fp32, kind="ExternalOutput")

    ms = _mom_coeffs(num_layers)

    with TileContext(nc) as tc:
        with (
            tc.tile_pool(name="wp", bufs=1) as wp,
            tc.tile_pool(name="st", bufs=1) as st,
            tc.tile_pool(name="scr", bufs=2) as scr,
            tc.tile_pool(name="scr1", bufs=1) as scr1,
            tc.tile_pool(name="ps_u", bufs=3, space="PSUM") as ps_u,
            tc.tile_pool(name="ps_g", bufs=3, space="PSUM") as ps_g,
            tc.tile_pool(name="ps_s", bufs=2, space="PSUM") as ps_s,
        ):
            bmat = wp.tile([128, 128], bf16, tag="bmat")
            eps = wp.tile([128, 1], fp32, tag="eps")
            nc.vector.memset(eps, 1e-30)

            class Sub:
                def __init__(self, q, bl, xt_d, w_d, wt_d, ot_d):
                    self.q = q
                    self.bl = bl
                    self.fd = NT * bl
                    self.nch = self.fd // CHUNK
                    self.tpc = CHUNK // bl  # d-tiles per chunk
                    self.sgp = CHUNK // bl  # s-tiles per pu bank
                    self.npu = NS // self.sgp
                    self.ot_d = ot_d
                    self.wsb = wp.tile([128, NT, N], bf16, tag=f"wsb{q}", name=f"wsb{q}")
                    self.wtsb = wp.tile([128, NS, D], bf16, tag=f"wtsb{q}", name=f"wtsb{q}")
                    self.xts = wp.tile([128, NS, bl], bf16, tag=f"xts{q}", name=f"xts{q}")
                    self.dma_w = lambda: nc.sync.dma_start(
                        out=self.wsb, in_=w_d.rearrange("(t p) n -> p t n", p=128))
                    # per-s-tile DMAs so the first y2 matmuls only wait on
                    # the first 512KB slice, not the full 4MB transfer
                    self.dma_wt = lambda: [nc.sync.dma_start(
                        out=self.wtsb[:, s, :],
                        in_=wt_d.rearrange("(s p) e -> p s e", p=128)[:, s, :])
                        for s in range(NS)]
                    self.dma_x = lambda: nc.sync.dma_start(
                        out=self.xts, in_=xt_d.rearrange("(s p) b -> p s b", p=128))
                    self.y2 = st.tile([128, self.fd], bf16, tag=f"y2{q}", name=f"y2{q}")
                    self.xb = [st.tile([128, self.fd], fp32, tag=f"xb{i}{q}",
                                       name=f"xb{i}{q}") for i in range(2)]
                    self.uT = st.tile([128, NS * bl], bf16, tag=f"uT{q}", name=f"uT{q}")
                    self.xtmp = [st.tile([128, CHUNK], bf16, tag=f"xtmp{j}{q}",
                                         name=f"xtmp{j}{q}") for j in range(self.nch)]
                    self.pre = [st.tile([128, CHUNK], fp32, tag=f"pre{j}{q}",
                                        name=f"pre{j}{q}") for j in range(self.nch)]
                    nc.vector.memset(self.xb[0], 0.0)

                def y2_phase(self):
                    bl, tpc = self.bl, self.tpc
                    for j in range(self.nch):
                        py = ps_g.tile([128, CHUNK], fp32, tag="pg")
                        for tt in range(tpc):
                            t = j * tpc + tt
                            for s in range(NS):
                                nc.tensor.matmul(
                                    py[:, tt * bl:(tt + 1) * bl],
                                    self.wtsb[:, s, t * 128:(t + 1) * 128],
                                    self.xts[:, s, :],
                                    start=(tt == 0 and s == 0),
                                    stop=(tt == tpc - 1 and s == NS - 1))
                        nc.scalar.mul(
                            self.y2[:, j * CHUNK:(j + 1) * CHUNK], py, TAU)

                def mm_phase(self, k):
                    """u-phase + grad-phase + v-combine (v written in-place
                    into pre). Emitted t-major so u matmuls for d-chunk j
                    only wait on xtmp[j]."""
                    bl, tpc, sgp = self.bl, self.tpc, self.sgp
                    pus = [ps_u.tile([128, CHUNK], fp32, tag="pu",
                                     name=f"pu{self.q}{p}") for p in range(self.npu)]
                    for t in range(NT):
                        for s in range(NS):
                            nc.tensor.matmul(
                                pus[s // sgp][:, (s % sgp) * bl:(s % sgp + 1) * bl],
                                self.wsb[:, t, s * 128:(s + 1) * 128],
                                self.xtmp[t // tpc][:, (t % tpc) * bl:(t % tpc + 1) * bl],
                                start=(t == 0 and s % sgp == 0),
                                stop=(t == NT - 1 and s % sgp == sgp - 1))
                    for p in range(self.npu):
                        nc.scalar.copy(
                            self.uT[:, p * CHUNK:(p + 1) * CHUNK], pus[p])
                    for j in range(self.nch):
                        pg = ps_g.tile([128, CHUNK], fp32, tag="pg")
                        for tt in range(tpc):
                            t = j * tpc + tt
                            for s in range(NS):
                                nc.tensor.matmul(
                                    pg[:, tt * bl:(tt + 1) * bl],
                                    self.wtsb[:, s, t * 128:(t + 1) * 128],
                                    self.uT[:, s * bl:(s + 1) * bl],
                                    start=(tt == 0 and s == 0),
                                    stop=(tt == tpc - 1 and s == NS - 1))
                        # v = pre - TAU*grad, in place (pre is rebuilt below)
                        nc.vector.scalar_tensor_tensor(
                            self.pre[j], pg, -TAU, self.pre[j],
                            op0=OP.mult, op1=OP.add)

                def act_phase(self, k):
                    """Group soft-threshold + momentum on each chunk.
                    k == 1 reads v from y2; else v is in pre (in-place)."""
                    xnew, xold = self.xb[k % 2], self.xb[(k - 1) % 2]
                    m = ms[k - 1]
                    last = k == num_layers
                    for j in range(self.nch):
                        sl = slice(j * CHUNK, (j + 1) * CHUNK)
                        vj = self.y2[:, sl] if k == 1 else self.pre[j][:, :]
                        v2 = scr1.tile([128, CHUNK], bf16, tag="v2")
                        nc.scalar.square(v2, vj)
                        gs = ps_s.tile([128, CHUNK], fp32, tag="gs")
                        nc.tensor.matmul(gs, bmat, v2, start=True, stop=True)
                        nrm = scr.tile([128, CHUNK], fp32, tag="nrm")
                        # +1e-30 guards reciprocal_approx_fast's undefined
                        # 0-input; relu(1 - CTH/1e-15) = 0 matches reference
                        nc.scalar.activation(nrm, gs, AF.Sqrt,
                                             bias=eps[:, :], scale=1.0)
                        invn = scr.tile([128, CHUNK], fp32, tag="invn")
                        nc.vector.reciprocal_approx_fast(invn, nrm)
                        scl = scr.tile([128, CHUNK], fp32, tag="scl")
                        nc.scalar.activation(scl, invn, AF.Relu,
                                             bias=1.0, scale=-CTH)
                        # xnew = max(v, 0) * scl
                        nc.vector.scalar_tensor_tensor(
                            xnew[:, sl], vj, 0.0, scl,
                            op0=OP.max, op1=OP.mult)
                        if not last:
                            dd = scr.tile([128, CHUNK], fp32, tag="dd")
                            nc.gpsimd.tensor_sub(dd, xnew[:, sl], xold[:, sl])
                            # bf16 matmul input: xtmp = xnew + m*dd
                            nc.vector.scalar_tensor_tensor(
                                self.xtmp[j], dd, m, xnew[:, sl],
                                op0=OP.mult, op1=OP.add)
                            # fp32 identity path: pre = (m*dd + xnew) + y2
                            nc.vector.scalar_tensor_tensor(
                                self.pre[j], dd, m, xnew[:, sl],
                                op0=OP.mult, op1=OP.add)
                            nc.gpsimd.tensor_add(self.pre[j], self.pre[j],
                                                 self.y2[:, sl])

                def decode(self):
                    bl, sgp = self.bl, self.sgp
                    z = self.xb[num_layers % 2]
                    zbf = st.tile([128, self.fd], bf16, tag="zbf")
                    nc.scalar.copy(zbf, z)
                    pds = [ps_u.tile([128, CHUNK], fp32, tag="pu",
                                     name=f"pd{self.q}{p}") for p in range(self.npu)]
                    for t in range(NT):
                        for s in range(NS):
                            nc.tensor.matmul(
                                pds[s // sgp][:, (s % sgp) * bl:(s % sgp + 1) * bl],
                                self.wsb[:, t, s * 128:(s + 1) * 128],
                                zbf[:, t * bl:(t + 1) * bl],
                                start=(t == 0 and s % sgp == 0),
                                stop=(t == NT - 1 and s % sgp == sgp - 1))
                    otsb = st.tile([128, NS, bl], fp32, tag="otsb")
                    for s in range(NS):
                        nc.scalar.copy(
                            otsb[:, s, :],
                            pds[s // sgp][:, (s % sgp) * bl:(s % sgp + 1) * bl])
                    nc.sync.dma_start(
                        out=self.ot_d.rearrange("(s p) b -> p s b", p=128),
                        in_=otsb)

            A = Sub("a", BLA, xta, wa, wta, ota)
            Bs = Sub("b", BLB, xtb, wb, wtb, otb)
            nc.sync.dma_start(out=bmat, in_=bm[:, :])
            A.dma_wt(); A.dma_x()
            Bs.dma_wt(); Bs.dma_x()
            A.dma_w(); Bs.dma_w()

            A.y2_phase(); Bs.y2_phase()
            A.act_phase(1); Bs.act_phase(1)
            # B's act is skewed half an iteration behind A's so its gs
            # matmuls never head the PE stream right behind B-grad (the
            # act chain then has A's matmul span to complete instead).
            A.mm_phase(2); Bs.mm_phase(2); A.act_phase(2)
            for k in range(3, num_layers + 1):
                A.mm_phase(k)
                Bs.act_phase(k - 1)
                Bs.mm_phase(k)
                A.act_phase(k)
            Bs.act_phase(num_layers)
            A.decode(); Bs.decode()

    nc.compile()
    return nc


_CACHED = {}


def _get_nc(num_layers=NUM_LAYERS):
    if num_layers not in _CACHED:
        _CACHED[num_layers] = build(num_layers)
    return _CACHED[num_layers]


def make_in_maps(x, w):
    """x [B,C,N] fp32, w [C,D,N] fp32 -> list of 8 per-core input dicts."""
    import ml_dtypes

    bf = ml_dtypes.bfloat16
    x = np.asarray(x, dtype=np.float32)
    w32 = np.ascontiguousarray(np.asarray(w, dtype=np.float32))
    wb_ = w32.astype(bf)
    wtb_ = np.ascontiguousarray(w32.transpose(0, 2, 1)).astype(bf)
    bmb = _bmat_np().astype(bf)
    maps = []
    for i in range(N_CORES):
        ca = 0 if i < 4 else 1
        ra = (i % 4) * BLA
        xa = np.ascontiguousarray(x[ra:ra + BLA, ca].T).astype(bf)  # [N, BLA]
        rb = i * BLB
        xb_ = np.ascontiguousarray(x[rb:rb + BLB, 2].T).astype(bf)  # [N, BLB]
        maps.append({
            "xta": xa, "wa": wb_[ca], "wta": wtb_[ca],
            "xtb": xb_, "wb": wb_[2], "wtb": wtb_[2], "bm": bmb,
        })
    return maps


def assemble_out(results):
    out = np.empty((B, C, N), np.float32)
    for i in range(N_CORES):
        ca = 0 if i < 4 else 1
        ra = (i % 4) * BLA
        out[ra:ra + BLA, ca] = results[i]["ota"].T  # [N, BLA] -> [BLA, N]
        rb = i * BLB
        out[rb:rb + BLB, 2] = results[i]["otb"].T
    return out


def kernel(x, W):
    from concourse.bass_utils import run_bass_kernel_spmd

    nc = _get_nc()
    res = run_bass_kernel_spmd(nc, make_in_maps(x, W), list(range(N_CORES)))
    return assemble_out(res.results)


if __name__ == "__main__":
    xs = np.random.randn(B, C, N).astype(np.float32)
    ws = np.random.randn(C, D, N).astype(np.float32)
    ws /= np.linalg.norm(ws, axis=-1, keepdims=True)
    out = kernel(xs, ws)
    print("out", out.shape, out.dtype, float(np.abs(out).mean()))


# revision 20
# speedup vs baseline: 9.9809x; 1.0508x over previous
"""GroupSparseAE (FISTA group-lasso encoder + linear decoder) on 8 trn2 cores.

Sharding (channel-major, 1536 (row, channel) atoms over 8 cores):
  sub A: cores 0-3 take channel 0 rows [128i, 128i+128); cores 4-7 take
         channel 1 rows likewise  -> 128-row block, one W.
  sub B: every core takes channel 2 rows [64i, 64i+64) -> 64-row block.
A's 128-wide moving operand halves the LDWEIGHTS-per-row cost vs 64-wide.

A and B are independent FISTA problems; their per-iteration pipelines are
interleaved (A-matmuls, B-matmuls, A-activation, B-activation) so each
sub's serial activation chain (square -> group-sum matmul -> sqrt -> recip
-> scale -> threshold -> momentum) hides under the other sub's matmuls.

Per sub, per iteration k = 1..30 with state in transposed [D, b] layout:
    u^T    = W^T-contract:  uT[n,b]   = sum_d W[d,n] xT[d,b]
    grad^T = gT[e,b]        = sum_n WT[n,e] uT[n,b]
    v      = xT_tmp + y2 - TAU*gT          (y2 = TAU * W @ x^T, precomputed)
    group soft-threshold (groups of 8 along d = partition dim):
       gs = Bmat^T @ v^2  (Bmat block-diag ones -> broadcast group sumsq)
       xnew = relu(v) * relu(1 - c/sqrt(gs))
    momentum: xtmp = xnew + m_k (xnew - xold)
  decode: out^T[n,b] = sum_d W[d,n] z[d,b]

Precision: all matmul operands bf16 (fp32 matmul = 2 half-speed passes);
the FISTA state (xnew/xold/pre) stays fp32 so quantization does not
accumulate over 30 iterations (numpy sim: 4.6e-3 rel err vs 1.7e-2 with
bf16 state). y2 is stored bf16 (constant perturbation only).

PSUM accumulators are [128, 512] (one bank, 8/4 interleaved chains); only
the first matmul into a bank uses start=True, because first_mm clears the
has_written bits of the WHOLE bank.
"""

import sys

sys.path.insert(0, "/opt/trn_rl_repo")

import numpy as np

B, C, N = 512, 3, 1024
G, S = 256, 8
D = G * S  # 2048
NUM_LAYERS = 30
TAU, LAM = 0.1, 0.1
CTH = LAM * TAU  # group threshold constant

N_CORES = 8
NT = D // 128  # 16 d-tiles
NS = N // 128  # 8 n-tiles
BLA = 128  # sub-A rows per core
BLB = 64  # sub-B rows per core
CHUNK = 512  # elementwise chunk, one PSUM bank of fp32


def _mom_coeffs(num_layers):
    # fp32 t-sequence to match the reference's on-device arithmetic
    one, four, two = np.float32(1.0), np.float32(4.0), np.float32(2.0)
    t = np.float32(1.0)
    ms = []
    for _ in range(num_layers):
        t_new = (one + np.sqrt(one + four * t * t)) / two
        ms.append(float((t - one) / t_new))
        t = t_new
    return ms


def _bmat_np():
    p = np.arange(128)
    return (p[:, None] // S == p[None, :] // S).astype(np.float32)


def build(num_layers=NUM_LAYERS):
    import concourse.bacc as bacc
    from concourse import mybir
    from concourse.tile import TileContext

    fp32 = mybir.dt.float32
    bf16 = mybir.dt.bfloat16
    AF = mybir.ActivationFunctionType
    OP = mybir.AluOpType

    nc = bacc.Bacc("TRN2", target_bir_lowering=False, debug=False,
                   num_devices=N_CORES)
    xta = nc.dram_tensor("xta", [N, BLA], bf16, kind="ExternalInput")
    wa = nc.dram_tensor("wa", [D, N], bf16, kind="ExternalInput")
    wta = nc.dram_tensor("wta", [N, D], bf16, kind="ExternalInput")
    xtb = nc.dram_tensor("xtb", [N, BLB], bf16, kind="ExternalInput")
    wb = nc.dram_tensor("wb", [D, N], bf16, kind="ExternalInput")
    wtb = nc.dram_tensor("wtb", [N, D], bf16, kind="ExternalInput")
    bm = nc.dram_tensor("bm", [128, 128], bf16, kind="ExternalInput")
    ota = nc.dram_tensor("ota", [N, BLA], fp32, kind="ExternalOutput")
    otb = nc.dram_tensor("otb", [N, BLB], fp32, kind="ExternalOutput")

    ms = _mom_coeffs(num_layers)

    with TileContext(nc) as tc:
        with (
            tc.tile_pool(name="wp", bufs=1) as wp,
            tc.tile_pool(name="st", bufs=1) as st,
            tc.tile_pool(name="scr", bufs=2) as scr,
            tc.tile_pool(name="scr1", bufs=1) as scr1,
            tc.tile_pool(name="ps_u", bufs=3, space="PSUM") as ps_u,
            tc.tile_pool(name="ps_g", bufs=3, space="PSUM") as ps_g,
            tc.tile_pool(name="ps_s", bufs=2, space="PSUM") as ps_s,
        ):
            bmat = wp.tile([128, 128], bf16, tag="bmat")
            eps = wp.tile([128, 1], fp32, tag="eps")
            nc.vector.memset(eps, 1e-30)

            class Sub:
                def __init__(self, q, bl, xt_d, w_d, wt_d, ot_d):
                    self.q = q
                    self.bl = bl
                    self.fd = NT * bl
                    self.nch = self.fd // CHUNK
                    self.tpc = CHUNK // bl  # d-tiles per chunk
                    self.sgp = CHUNK // bl  # s-tiles per pu bank
                    self.npu = NS // self.sgp
                    self.ot_d = ot_d
                    self.wsb = wp.tile([128, NT, N], bf16, tag=f"wsb{q}", name=f"wsb{q}")
                    self.wtsb = wp.tile([128, NS, D], bf16, tag=f"wtsb{q}", name=f"wtsb{q}")
                    self.xts = wp.tile([128, NS, bl], bf16, tag=f"xts{q}", name=f"xts{q}")
                    self.dma_w = lambda: nc.sync.dma_start(
                        out=self.wsb, in_=w_d.rearrange("(t p) n -> p t n", p=128))
                    # per-s-tile DMAs so the first y2 matmuls only wait on
                    # the first 512KB slice, not the full 4MB transfer
                    self.dma_wt = lambda: [nc.sync.dma_start(
                        out=self.wtsb[:, s, :],
                        in_=wt_d.rearrange("(s p) e -> p s e", p=128)[:, s, :])
                        for s in range(NS)]
                    self.dma_x = lambda: nc.sync.dma_start(
                        out=self.xts, in_=xt_d.rearrange("(s p) b -> p s b", p=128))
                    self.y2 = st.tile([128, self.fd], bf16, tag=f"y2{q}", name=f"y2{q}")
                    self.xb = [st.tile([128, self.fd], fp32, tag=f"xb{i}{q}",
                                       name=f"xb{i}{q}") for i in range(2)]
                    self.uT = st.tile([128, NS * bl], bf16, tag=f"uT{q}", name=f"uT{q}")
                    self.xtmp = [st.tile([128, CHUNK], bf16, tag=f"xtmp{j}{q}",
                                         name=f"xtmp{j}{q}") for j in range(self.nch)]
                    self.pre = [st.tile([128, CHUNK], fp32, tag=f"pre{j}{q}",
                                        name=f"pre{j}{q}") for j in range(self.nch)]
                    nc.vector.memset(self.xb[0], 0.0)

                def y2_phase(self):
                    bl, tpc = self.bl, self.tpc
                    for j in range(self.nch):
                        py = ps_g.tile([128, CHUNK], fp32, tag="pg")
                        for tt in range(tpc):
                            t = j * tpc + tt
                            for s in range(NS):
                                nc.tensor.matmul(
                                    py[:, tt * bl:(tt + 1) * bl],
                                    self.wtsb[:, s, t * 128:(t + 1) * 128],
                                    self.xts[:, s, :],
                                    start=(tt == 0 and s == 0),
                                    stop=(tt == tpc - 1 and s == NS - 1))
                        nc.scalar.mul(
                            self.y2[:, j * CHUNK:(j + 1) * CHUNK], py, TAU)

                def mm_phase(self, k):
                    """u-phase + grad-phase + v-combine (v written in-place
                    into pre). Emitted t-major so u matmuls for d-chunk j
                    only wait on xtmp[j]."""
                    bl, tpc, sgp = self.bl, self.tpc, self.sgp
                    pus = [ps_u.tile([128, CHUNK], fp32, tag="pu",
                                     name=f"pu{self.q}{p}") for p in range(self.npu)]
                    for t in range(NT):
                        for s in range(NS):
                            nc.tensor.matmul(
                                pus[s // sgp][:, (s % sgp) * bl:(s % sgp + 1) * bl],
                                self.wsb[:, t, s * 128:(s + 1) * 128],
                                self.xtmp[t // tpc][:, (t % tpc) * bl:(t % tpc + 1) * bl],
                                start=(t == 0 and s % sgp == 0),
                                stop=(t == NT - 1 and s % sgp == sgp - 1))
                    for p in range(self.npu):
                        # DVE copy: faster than ACT and keeps ACT free for
                        # the sqrt/scale chain this copy would queue behind
                        nc.vector.tensor_copy(
                            self.uT[:, p * CHUNK:(p + 1) * CHUNK], pus[p])
                    for j in range(self.nch):
                        pg = ps_g.tile([128, CHUNK], fp32, tag="pg")
                        for tt in range(tpc):
                            t = j * tpc + tt
                            for s in range(NS):
                                nc.tensor.matmul(
                                    pg[:, tt * bl:(tt + 1) * bl],
                                    self.wtsb[:, s, t * 128:(t + 1) * 128],
                                    self.uT[:, s * bl:(s + 1) * bl],
                                    start=(tt == 0 and s == 0),
                                    stop=(tt == tpc - 1 and s == NS - 1))
                        # v = pre - TAU*grad, in place (pre is rebuilt below)
                        nc.vector.scalar_tensor_tensor(
                            self.pre[j], pg, -TAU, self.pre[j],
                            op0=OP.mult, op1=OP.add)

                def act_phase(self, k):
                    """Group soft-threshold + momentum on each chunk.
                    k == 1 reads v from y2; else v is in pre (in-place)."""
                    xnew, xold = self.xb[k % 2], self.xb[(k - 1) % 2]
                    m = ms[k - 1]
                    last = k == num_layers
                    for j in range(self.nch):
                        sl = slice(j * CHUNK, (j + 1) * CHUNK)
                        vj = self.y2[:, sl] if k == 1 else self.pre[j][:, :]
                        v2 = scr1.tile([128, CHUNK], bf16, tag="v2")
                        nc.scalar.square(v2, vj)
                        gs = ps_s.tile([128, CHUNK], fp32, tag="gs")
                        nc.tensor.matmul(gs, bmat, v2, start=True, stop=True)
                        nrm = scr.tile([128, CHUNK], fp32, tag="nrm")
                        # +1e-30 guards reciprocal_approx_fast's undefined
                        # 0-input; relu(1 - CTH/1e-15) = 0 matches reference
                        nc.scalar.activation(nrm, gs, AF.Sqrt,
                                             bias=eps[:, :], scale=1.0)
                        invn = scr.tile([128, CHUNK], fp32, tag="invn")
                        nc.vector.reciprocal_approx_fast(invn, nrm)
                        scl = scr.tile([128, CHUNK], fp32, tag="scl")
                        nc.scalar.activation(scl, invn, AF.Relu,
                                             bias=1.0, scale=-CTH)
                        # xnew = max(v, 0) * scl
                        nc.vector.scalar_tensor_tensor(
                            xnew[:, sl], vj, 0.0, scl,
                            op0=OP.max, op1=OP.mult)
                        if not last:
                            dd = scr.tile([128, CHUNK], fp32, tag="dd")
                            nc.gpsimd.tensor_sub(dd, xnew[:, sl], xold[:, sl])
                            # bf16 matmul input: xtmp = xnew + m*dd
                            nc.vector.scalar_tensor_tensor(
                                self.xtmp[j], dd, m, xnew[:, sl],
                                op0=OP.mult, op1=OP.add)
                            # fp32 identity path: pre = (m*dd + xnew) + y2
                            nc.vector.scalar_tensor_tensor(
                                self.pre[j], dd, m, xnew[:, sl],
                                op0=OP.mult, op1=OP.add)
                            nc.gpsimd.tensor_add(self.pre[j], self.pre[j],
                                                 self.y2[:, sl])

                def decode(self):
                    bl, sgp = self.bl, self.sgp
                    z = self.xb[num_layers % 2]
                    zbf = st.tile([128, self.fd], bf16, tag="zbf")
                    nc.scalar.copy(zbf, z)
                    pds = [ps_u.tile([128, CHUNK], fp32, tag="pu",
                                     name=f"pd{self.q}{p}") for p in range(self.npu)]
                    for t in range(NT):
                        for s in range(NS):
                            nc.tensor.matmul(
                                pds[s // sgp][:, (s % sgp) * bl:(s % sgp + 1) * bl],
                                self.wsb[:, t, s * 128:(s + 1) * 128],
                                zbf[:, t * bl:(t + 1) * bl],
                                start=(t == 0 and s % sgp == 0),
                                stop=(t == NT - 1 and s % sgp == sgp - 1))
                    otsb = st.tile([128, NS, bl], fp32, tag="otsb")
                    for s in range(NS):
                        nc.scalar.copy(
                            otsb[:, s, :],
                            pds[s // sgp][:, (s % sgp) * bl:(s % sgp + 1) * bl])
                    nc.sync.dma_start(
                        out=self.ot_d.rearrange("(s p) b -> p s b", p=128),
                        in_=otsb)

            A = Sub("a", BLA, xta, wa, wta, ota)
            Bs = Sub("b", BLB, xtb, wb, wtb, otb)
            nc.sync.dma_start(out=bmat, in_=bm[:, :])
            A.dma_wt(); A.dma_x()
            Bs.dma_wt(); Bs.dma_x()
            A.dma_w(); Bs.dma_w()

            A.y2_phase(); Bs.y2_phase()
            A.act_phase(1); Bs.act_phase(1)
            # B's act is skewed half an iteration behind A's so its gs
            # matmuls never head the PE stream right behind B-grad (the
            # act chain then has A's matmul span to complete instead).
            A.mm_phase(2); Bs.mm_phase(2); A.act_phase(2)
            for k in range(3, num_layers + 1):
                A.mm_phase(k)
                Bs.act_phase(k - 1)
                Bs.mm_phase(k)
                A.act_phase(k)
            Bs.act_phase(num_layers)
            A.decode(); Bs.decode()

    nc.compile()
    return nc


_CACHED = {}


def _get_nc(num_layers=NUM_LAYERS):
    if num_layers not in _CACHED:
        _CACHED[num_layers] = build(num_layers)
    return _CACHED[num_layers]


def make_in_maps(x, w):
    """x [B,C,N] fp32, w [C,D,N] fp32 -> list of 8 per-core input dicts."""
    import ml_dtypes

    bf = ml_dtypes.bfloat16
    x = np.asarray(x, dtype=np.float32)
    w32 = np.ascontiguousarray(np.asarray(w, dtype=np.float32))
    wb_ = w32.astype(bf)
    wtb_ = np.ascontiguousarray(w32.transpose(0, 2, 1)).astype(bf)
    bmb = _bmat_np().astype(bf)
    maps = []
    for i in range(N_CORES):
        ca = 0 if i < 4 else 1
        ra = (i % 4) * BLA
        xa = np.ascontiguousarray(x[ra:ra + BLA, ca].T).astype(bf)  # [N, BLA]
        rb = i * BLB
        xb_ = np.ascontiguousarray(x[rb:rb + BLB, 2].T).astype(bf)  # [N, BLB]
        maps.append({
            "xta": xa, "wa": wb_[ca], "wta": wtb_[ca],
            "xtb": xb_, "wb": wb_[2], "wtb": wtb_[2], "bm": bmb,
        })
    return maps


def assemble_out(results):
    out = np.empty((B, C, N), np.float32)
    for i in range(N_CORES):
        ca = 0 if i < 4 else 1
        ra = (i % 4) * BLA
        out[ra:ra + BLA, ca] = results[i]["ota"].T  # [N, BLA] -> [BLA, N]
        rb = i * BLB
        out[rb:rb + BLB, 2] = results[i]["otb"].T
    return out


def kernel(x, W):
    from concourse.bass_utils import run_bass_kernel_spmd

    nc = _get_nc()
    res = run_bass_kernel_spmd(nc, make_in_maps(x, W), list(range(N_CORES)))
    return assemble_out(res.results)


if __name__ == "__main__":
    xs = np.random.randn(B, C, N).astype(np.float32)
    ws = np.random.randn(C, D, N).astype(np.float32)
    ws /= np.linalg.norm(ws, axis=-1, keepdims=True)
    out = kernel(xs, ws)
    print("out", out.shape, out.dtype, float(np.abs(out).mean()))
